# revision 57
# baseline (speedup 1.0000x reference)
"""Trainium2 Bass kernel for nn_ConditioningEncoder (6-layer attention encoder).

Strategy: data-parallel over batch (B=8 -> 1 batch element per NeuronCore).
All big matmuls run in bf16; f32 accumulation in PSUM throughout.

Per-core computation (C=1024 channels, L=1024 positions, 16 heads, dh=64):
  x = init_w @ speech + init_b                        [C, L] f32
  6x attention blocks:
    h = GroupNorm32(x) * gn_w + gn_b                  (bf16, batched stats)
    k, v = projections of h (head-major channel order)
    vT = PE-transpose of v per head pair + ones column (denominator trick)
    per head-pair p (keeps PE full-width / HAM warm):
      q-projection for chunk p (full 128x128 matmuls)
      chains (t-block): S^T for BOTH heads of the pair issued back-to-back
        as concurrent row-tiled matmuls (tile_position (0,0)/(64,0)) into a
        2-bank PSUM tile; ONE exp ACTIVATE (N=1024) -> et (bf16)
      pav[65,2,t] = [v;1]^T E accumulated over s-chunks (PE), per head
      av_out: DVE copies attn + den rows; den DMA -> [16, L] tile
      pass-2 per pair: rec=1/den (DVE), broadcast via K=2 sel matmul,
        recb copy (DVE), attn *= recb (gpsimd)
    x += proj_w @ attn + proj_b
  out = x[:, 0]

Layer 5 computes q/attention/proj only for t=0..16 (only column 0 returned),
with all 16 S matmuls of a pair batched into one PSUM tile + one exp.
"""
import sys

sys.path.insert(0, "/opt/trn_rl_repo")

from contextlib import ExitStack

import numpy as np
import ml_dtypes

import os

import concourse.bass as bass
import concourse.tile as tile
from concourse import bacc, mybir
from concourse.bass_utils import run_bass_kernel_spmd
from concourse.tile import add_dep_helper

# NOTE: S matmuls with lhsT/rhs at partition base 64 (auto tile_position
# (64,0)) abort on HW when issued adjacent to (0,0) matmuls — instead k is
# staged as two zero-padded copies so every S matmul is a full 128x128
# (K=128 with 64 zero rows; same streaming cost, no tiling modes).
K_SPLIT_EXP = bool(int(os.environ.get("K_SPLIT_EXP", "0")))
K_SPLIT_DMA = bool(int(os.environ.get("K_SPLIT_DMA", "0")))
K_NO_PAIRS = bool(int(os.environ.get("K_NO_PAIRS", "0")))
K_L5_OLD = bool(int(os.environ.get("K_L5_OLD", "0")))

f32 = mybir.dt.float32
f32r = mybir.dt.float32r
bf16 = mybir.dt.bfloat16
AF = mybir.ActivationFunctionType
Alu = mybir.AluOpType

B, SPEC, L = 8, 80, 1024
C, H, DH, NL, NG = 1024, 16, 64, 6, 32
CC = C // 128          # channel chunks per full width
EPS = 1e-5
NCORES = 8

LAST_RESULT = None     # test harness reads exec_time from here
_CACHE = {}


def _build():
    nc = bacc.Bacc("TRN2", target_bir_lowering=False, debug=False,
                   num_devices=NCORES)

    dr = {}
    def din(name, shape, dt):
        dr[name] = nc.dram_tensor(name, shape, dt, kind="ExternalInput").ap()

    din("speech", [SPEC, L], bf16)
    din("initw", [SPEC, CC, 128], bf16)
    din("initb", [128, CC], f32)
    for w in ("wq", "wk", "wv", "wp"):
        din(w, [NL, CC, 128, CC, 128], bf16)
    for b in ("bq", "bk", "bv", "bp", "gnw", "gnb"):
        din(b, [128, NL, CC], f32)
    din("ind", [128, 4], f32r)
    din("indt", [4, 128], f32r)
    din("ident", [128, 128], bf16)
    din("vtinit", [128, CC, 8, 2, 65], bf16)
    din("sel2", [2, 128], bf16)
    din("epsc", [128, 1], f32)
    out_d = nc.dram_tensor("out", [C], f32, kind="ExternalOutput").ap()

    with tile.TileContext(nc) as tc, ExitStack() as ctx:
        cst = ctx.enter_context(tc.tile_pool(name="cst", bufs=1))
        big = ctx.enter_context(tc.tile_pool(name="big", bufs=1))
        wsp = ctx.enter_context(tc.tile_pool(name="wsp", bufs=2))
        ep = ctx.enter_context(tc.tile_pool(name="ep", bufs=2))
        sml = ctx.enter_context(tc.tile_pool(name="sml", bufs=2))
        ps = ctx.enter_context(tc.tile_pool(name="ps", bufs=1, space="PSUM"))

        # ---- constants ----
        ind = cst.tile([128, 4], f32r)
        nc.sync.dma_start(out=ind, in_=dr["ind"])
        indt = cst.tile([4, 128], f32r)
        nc.sync.dma_start(out=indt, in_=dr["indt"])
        ident = cst.tile([128, 128], bf16)
        nc.sync.dma_start(out=ident, in_=dr["ident"])
        sel2 = cst.tile([2, 128], bf16)
        nc.sync.dma_start(out=sel2, in_=dr["sel2"])
        biases = {}
        for b in ("bq", "bk", "bv", "bp", "gnw", "gnb"):
            t = cst.tile([128, NL, CC], f32, name=f"c_{b}")
            nc.sync.dma_start(out=t, in_=dr[b])
            biases[b] = t
        epsc = cst.tile([128, 1], f32)
        nc.sync.dma_start(out=epsc, in_=dr["epsc"])
        initb = cst.tile([128, CC], f32)
        nc.sync.dma_start(out=initb, in_=dr["initb"])

        # persistent activations
        x = big.tile([128, CC, L], f32)
        # vT[s, sc, pair, half, 0:64] = v chans; [..., 64] = 1.0 (denom col)
        vT = big.tile([128, CC, 8, 2, 65], bf16)
        nc.sync.dma_start(out=vT, in_=dr["vtinit"])
        # k staged as two zero-padded copies: kzA rows 64:128 and kzB rows
        # 0:64 stay zero forever, so S matmuls contract over a full K=128.
        kzA = big.tile([128, CC, L], bf16, name="kzA")
        kzB = big.tile([128, CC, L], bf16, name="kzB")
        nc.vector.memset(kzA[64:128, :, :], 0.0)
        nc.vector.memset(kzB[0:64, :, :], 0.0)

        # ---- init conv1x1 ----
        spt = cst.tile([SPEC, L], bf16, name="spt")
        nc.sync.dma_start(out=spt, in_=dr["speech"])
        iwt = cst.tile([SPEC, CC, 128], bf16, name="iwt")
        nc.sync.dma_start(out=iwt, in_=dr["initw"])
        for mc in range(CC):
            pm = ps.tile([128, 2, 512], f32, tag="s", bufs=2, name=f"pi{mc}")
            for tp in range(2):
                nc.tensor.matmul(pm[:, tp, :], lhsT=iwt[:, mc, :],
                                 rhs=spt[:, tp * 512:(tp + 1) * 512],
                                 start=True, stop=True)
            for tp in range(2):
                nc.vector.tensor_scalar(
                    x[:, mc, tp * 512:(tp + 1) * 512], pm[:, tp, :],
                    initb[:, mc:mc + 1], None, Alu.add)

        # ---- layers ----
        for l in range(NL):
            last = (l == NL - 1)
            TW = 16 if last else L     # t-width for q/attn/proj
            TB = 16 if last else 512   # attention t-block

            # GroupNorm: x -> h (bf16), stats batched across chunks
            h = big.tile([128, CC, L], bf16, tag="h", name=f"h{l}")
            st_all = sml.tile([128, CC, 2, 6], f32, tag="bst", bufs=1,
                              name=f"st{l}")
            for cc in range(CC):
                for u in range(2):
                    nc.vector.bn_stats(st_all[:, cc, u, :],
                                       x[:, cc, u * 512:(u + 1) * 512])
            mv = sml.tile([128, CC, 2], f32, tag="mv", bufs=1, name=f"mv{l}")
            for cc in range(CC):
                nc.vector.bn_aggr(mv[:, cc, :], st_all[:, cc, :, :])
            # grhs[:, cc, :] = [mean, var + mean^2]  (f32r)
            grhs = sml.tile([128, CC, 2], f32r, tag="grhs", bufs=1,
                            name=f"gr{l}")
            nc.vector.tensor_copy(out=grhs[:, :, 0], in_=mv[:, :, 0])
            sq = sml.tile([128, CC], f32, tag="sq", bufs=1, name=f"sq{l}")
            nc.vector.tensor_tensor(sq, mv[:, :, 0], mv[:, :, 0], Alu.mult)
            nc.vector.tensor_tensor(grhs[:, :, 1], mv[:, :, 1], sq, Alu.add)
            pg = ps.tile([4, CC, 2], f32, tag="s", bufs=2, name=f"pg{l}")
            nc.tensor.matmul(pg, lhsT=ind, rhs=grhs, start=True, stop=True)
            # group stats -> gmrhs [4, {mean, rstd}, CC] (f32r)
            gmrhs = sml.tile([4, 2, CC], f32r, tag="gm", bufs=1,
                             name=f"gm{l}")
            nc.vector.tensor_scalar(gmrhs[:, 0, :], pg[:, :, 0],
                                    1.0 / 32, None, Alu.mult)
            ex2 = sml.tile([4, CC], f32, tag="ex2", bufs=1, name=f"ex{l}")
            nc.vector.tensor_scalar(ex2, pg[:, :, 1], 1.0 / 32, None,
                                    Alu.mult)
            m2 = sml.tile([4, CC], f32, tag="m2", bufs=1, name=f"m2{l}")
            nc.vector.tensor_tensor(m2, gmrhs[:, 0, :], gmrhs[:, 0, :],
                                    Alu.mult)
            var = sml.tile([4, CC], f32, tag="var", bufs=1, name=f"va{l}")
            nc.vector.tensor_tensor(var, ex2, m2, Alu.subtract)
            sd = sml.tile([4, CC], f32, tag="sd", bufs=1, name=f"sd{l}")
            nc.scalar.activation(sd, var, AF.Sqrt, bias=epsc[0:4, :])
            with nc.allow_low_precision(reason="f32r rstd"):
                nc.vector.reciprocal(gmrhs[:, 1, :], sd)
            pb = ps.tile([128, 2, CC], f32, tag="s", bufs=2, name=f"pb{l}")
            nc.tensor.matmul(pb, lhsT=indt, rhs=gmrhs, start=True, stop=True)
            # scale = rstd*gnw ; shift = gnb - mean*scale
            sc_all = sml.tile([128, CC], f32, tag="scl", bufs=1,
                              name=f"sc{l}")
            nc.vector.tensor_tensor(sc_all, pb[:, 1, :],
                                    biases["gnw"][:, l, :], Alu.mult)
            ms = sml.tile([128, CC], f32, tag="ms", bufs=1, name=f"ms{l}")
            nc.vector.tensor_tensor(ms, pb[:, 0, :], sc_all, Alu.mult)
            sh_all = sml.tile([128, CC], f32, tag="sh", bufs=1, name=f"sh{l}")
            nc.vector.tensor_tensor(sh_all, biases["gnb"][:, l, :], ms,
                                    Alu.subtract)
            for cc in range(CC):
                # gpsimd: DVE is saturated with stats/evacs at layer start
                nc.gpsimd.tensor_scalar(h[:, cc, :], x[:, cc, :],
                                        sc_all[:, cc:cc + 1],
                                        sh_all[:, cc:cc + 1],
                                        Alu.mult, Alu.add)

            # k / v projections (v lands in the attn tile; vT extracted below)
            attn = big.tile([128, CC, L], bf16, tag="avb", name=f"av{l}")
            v = attn
            for dst, w, bias in ((None, "wk", "bk"), (v, "wv", "bv")):
                for mc in range(CC):
                    ws = wsp.tile([128, CC, 128], bf16, tag="ws",
                                  name=f"w{l}_{w}_{mc}")
                    nc.sync.dma_start(out=ws, in_=dr[w][l, mc])
                    pm = ps.tile([128, 2, 512], f32, tag="s", bufs=2,
                                 name=f"p{l}_{w}_{mc}")
                    for tp in range(2):
                        for kc in range(CC):
                            nc.tensor.matmul(pm[:, tp, :], lhsT=ws[:, kc, :],
                                             rhs=h[:, kc,
                                                   tp * 512:(tp + 1) * 512],
                                             start=(kc == 0),
                                             stop=(kc == CC - 1))
                    for tp in range(2):
                        tsl = slice(tp * 512, (tp + 1) * 512)
                        # evacuate on ACT: it idles during the k/v phase
                        # while DVE is saturated with stats/copies
                        if dst is None:   # k: split halves into kzA / kzB
                            nc.scalar.activation(
                                kzA[0:64, mc, tsl], pm[0:64, tp, :],
                                AF.Identity,
                                bias=biases[bias][0:64, l, mc:mc + 1])
                            nc.scalar.activation(
                                kzB[64:128, mc, tsl], pm[64:128, tp, :],
                                AF.Identity,
                                bias=biases[bias][64:128, l, mc:mc + 1])
                        else:
                            nc.scalar.activation(
                                dst[:, mc, tsl], pm[:, tp, :], AF.Identity,
                                bias=biases[bias][:, l, mc:mc + 1])

            # vT: transpose v per head pair; 2 s-chunks per PSUM tile, DVE evac
            for p8 in range(8):
                for sc2 in range(4):
                    pt = ps.tile([128, 2, 2, 64], bf16, tag="s", bufs=2,
                                 name=f"pt{l}_{p8}_{sc2}")
                    for j in range(2):
                        sc = sc2 * 2 + j
                        nc.tensor.transpose(
                            pt[:, j, :, :],
                            v[:, p8, sc * 128:(sc + 1) * 128], ident)
                    nc.scalar.copy(
                        out=vT[:, sc2 * 2:sc2 * 2 + 2, p8, :, 0:64], in_=pt)

            # attention: per head-pair q-projection + S/exp/AV chains
            q = big.tile([128, CC, TW], bf16, tag="q", name=f"q{l}")
            den2 = {}   # pair -> [2, TW] denominator tile (partition base 0)

            def av_mm_one(p8, t0, et, pav, sc, a):
                nc.tensor.matmul(pav[:, a, :],
                                 lhsT=vT[:, sc, p8, a, :],
                                 rhs=et[:, sc, a, :],
                                 start=(sc == 0), stop=(sc == CC - 1),
                                 skip_group_check=True)

            def av_mm(p8, t0, et, pav, sc):
                for a in range(2):
                    av_mm_one(p8, t0, et, pav, sc, a)

            def av_out(p8, t0, pav):
                nc.vector.tensor_copy(
                    out=attn[0:64, p8, t0:t0 + TB], in_=pav[0:64, 0, :])
                nc.vector.tensor_copy(
                    out=attn[64:128, p8, t0:t0 + TB], in_=pav[0:64, 1, :])
                stg = sml.tile([65, 2, TB], f32, tag="stg", bufs=2,
                               name=f"sg{l}_{p8}_{t0}")
                nc.vector.tensor_copy(out=stg[64:65, :, :],
                                      in_=pav[64:65, :, :])
                nc.sync.dma_start(out=den2[p8][:, t0:t0 + TB],
                                  in_=stg[64:65, :, :])

            def pass2(p8):
                # fast approx reciprocal (~18 bits; result is bf16'd anyway)
                r32 = sml.tile([2, TW], f32, tag="r32", bufs=2,
                               name=f"r32{l}_{p8}")
                nc.vector.reciprocal_approx_fast(out=r32, in_=den2[p8])
                rec = sml.tile([2, TW], bf16, tag="rec", bufs=2,
                               name=f"rc{l}_{p8}")
                nc.vector.tensor_copy(out=rec, in_=r32)
                for t0 in range(0, TW, TB):
                    pbc = ps.tile([128, TB], f32, tag="avp", bufs=2,
                                  name=f"pbc{l}_{p8}_{t0}")
                    nc.tensor.matmul(pbc, lhsT=sel2,
                                     rhs=rec[:, t0:t0 + TB],
                                     start=True, stop=True)
                    recb = sml.tile([128, TB], bf16, tag="recb", bufs=2,
                                    name=f"rb{l}_{p8}_{t0}")
                    nc.vector.tensor_copy(out=recb, in_=pbc)
                    nc.gpsimd.tensor_tensor(
                        attn[:, p8, t0:t0 + TB],
                        attn[:, p8, t0:t0 + TB],
                        recb, Alu.mult)

            prev = None
            for p8 in range(8):
                # q projection for this pair's chunk (full-array matmuls)
                wq = wsp.tile([128, CC, 128], bf16, tag="ws",
                              name=f"w{l}_q_{p8}")
                nc.sync.dma_start(out=wq, in_=dr["wq"][l, p8])
                pmq = ps.tile([128, 2, 512] if TW > 16 else [128, 2, 16],
                              f32, tag="s", bufs=2, name=f"pq{l}_{p8}")
                for tp in range(TW // 512 if TW > 16 else 1):
                    tw = min(512, TW)
                    for kc in range(CC):
                        nc.tensor.matmul(
                            pmq[:, tp, :], lhsT=wq[:, kc, :],
                            rhs=h[:, kc, tp * 512:tp * 512 + tw],
                            start=(kc == 0), stop=(kc == CC - 1))
                    nc.vector.tensor_scalar(
                        q[:, p8, tp * 512:tp * 512 + tw], pmq[:, tp, :],
                        biases["bq"][:, l, p8:p8 + 1], None, Alu.add)

                for t0 in range(0, TW, TB):
                    if t0 == 0:
                        den2[p8] = sml.tile([2, TW], f32, tag="den2",
                                            bufs=2, name=f"dn{l}_{p8}")
                    et = ep.tile([128, CC, 2, TB], bf16, tag="E", bufs=2,
                                 name=f"e{l}_{p8}_{t0}")
                    pav = (ps.tile([65, 2, TB], f32, tag="avp", bufs=2,
                                   name=f"pa{l}_{prev[0]}_{prev[1]}")
                           if prev is not None else None)
                    def s_lhs_rhs(a, sc):
                        kz = kzA if a == 0 else kzB
                        return (kz[:, p8, sc * 128:(sc + 1) * 128],
                                q[:, p8, t0:t0 + TB])

                    if TB == 16 and not K_L5_OLD:
                        # layer 5: all 16 S matmuls -> one PSUM tile, one exp
                        pss = ps.tile([128, CC, 2, TB], f32, tag="s", bufs=2,
                                      name=f"ps{l}_{p8}_{t0}")
                        for sc in range(CC):
                            for a in range(2):
                                lh, rh = s_lhs_rhs(a, sc)
                                nc.tensor.matmul(
                                    pss[:, sc, a, :], lhsT=lh, rhs=rh,
                                    start=True, stop=True)
                                if K_NO_PAIRS and prev is not None:
                                    av_mm_one(prev[0], prev[1], prev[2],
                                              pav, sc, a)
                            if not K_NO_PAIRS and prev is not None:
                                av_mm(prev[0], prev[1], prev[2], pav, sc)
                        nc.scalar.activation(et, pss, AF.Exp,
                                             bias=0.0, scale=0.125)
                    else:
                        for sc in range(CC):
                            pss = ps.tile([128, 2, TB], f32, tag="s", bufs=2,
                                          name=f"ps{l}_{p8}_{t0}_{sc}")
                            for a in range(2):
                                lh, rh = s_lhs_rhs(a, sc)
                                nc.tensor.matmul(
                                    pss[:, a, :], lhsT=lh, rhs=rh,
                                    start=True, stop=True)
                                if K_NO_PAIRS and prev is not None:
                                    av_mm_one(prev[0], prev[1], prev[2],
                                              pav, sc, a)
                            if K_SPLIT_EXP:
                                for a in range(2):
                                    nc.scalar.activation(
                                        et[:, sc, a, :], pss[:, a, :],
                                        AF.Exp, bias=0.0, scale=0.125)
                            else:
                                nc.scalar.activation(et[:, sc, :, :], pss,
                                                     AF.Exp, bias=0.0,
                                                     scale=0.125)
                            if not K_NO_PAIRS and prev is not None:
                                av_mm(prev[0], prev[1], prev[2], pav, sc)
                    if prev is not None:
                        av_out(prev[0], prev[1], pav)
                        if prev[1] + TB >= TW:
                            pass2(prev[0])
                    prev = (p8, t0, et)
            pav = ps.tile([65, 2, TB], f32, tag="avp", bufs=2,
                          name=f"pa{l}_{prev[0]}_{prev[1]}")
            for sc in range(CC):
                av_mm(prev[0], prev[1], prev[2], pav, sc)
            av_out(prev[0], prev[1], pav)
            pass2(prev[0])

            # proj + residual
            for mc in range(CC):
                ws = wsp.tile([128, CC, 128], bf16, tag="ws",
                              name=f"wp{l}_{mc}")
                nc.sync.dma_start(out=ws, in_=dr["wp"][l, mc])
                pm = ps.tile([128, 2, 512] if TW > 16 else [128, 2, 16],
                             f32, tag="s", bufs=2, name=f"pp{l}_{mc}")
                for t0 in range(0, TW, 512):
                    tw = min(512, TW - t0)
                    tp = t0 // 512
                    for kc in range(CC):
                        nc.tensor.matmul(pm[:, tp, :], lhsT=ws[:, kc, :],
                                         rhs=attn[:, kc, t0:t0 + tw],
                                         start=(kc == 0), stop=(kc == CC - 1))
                    nc.vector.scalar_tensor_tensor(
                        out=x[:, mc, t0:t0 + tw], in0=pm[:, tp, :],
                        scalar=biases["bp"][:, l, mc:mc + 1],
                        in1=x[:, mc, t0:t0 + tw], op0=Alu.add, op1=Alu.add)

        # ---- output: x[:, :, 0] ----
        o = cst.tile([128, CC], f32)
        nc.vector.tensor_copy(out=o, in_=x[:, :, 0:1].squeeze(-1))
        nc.sync.dma_start(out=out_d.rearrange("(c p) -> p c", p=128), in_=o)

    nc.compile()
    return nc


def _vtinit(bf):
    a = np.zeros((128, CC, 8, 2, 65), np.float32)
    a[:, :, :, :, 64] = 1.0
    return a.astype(bf)


def _prep(inputs):
    """Host-side weight restaging -> per-core input maps."""
    g = {k: np.asarray(v, np.float32) for k, v in inputs.items()}
    bf = ml_dtypes.bfloat16

    idx = np.arange(3 * C).reshape(H, 3, DH)
    qidx, kidx, vidx = idx[:, 0].ravel(), idx[:, 1].ravel(), idx[:, 2].ravel()

    def stage_w(w):            # w [NL, 1024(out), 1024(in)] -> staged lhsT
        wt = w.transpose(0, 2, 1)                    # [l, in, out]
        return np.ascontiguousarray(
            wt.reshape(NL, CC, 128, CC, 128).transpose(0, 3, 2, 1, 4)
        ).astype(bf)

    def stage_b(b):            # [NL, 1024] -> [128, NL, CC]
        return np.ascontiguousarray(
            b.reshape(NL, CC, 128).transpose(2, 0, 1))

    sel2 = np.zeros((2, 128), np.float32)
    for p in range(2):
        sel2[p, p * 64:(p + 1) * 64] = 1.0

    qkv_w, qkv_b = g["qkv_w"], g["qkv_b"]
    common = {
        "wq": stage_w(qkv_w[:, qidx, :]),
        "wk": stage_w(qkv_w[:, kidx, :]),
        "wv": stage_w(qkv_w[:, vidx, :]),
        "wp": stage_w(g["proj_w"]),
        "bq": stage_b(qkv_b[:, qidx]),
        "bk": stage_b(qkv_b[:, kidx]),
        "bv": stage_b(qkv_b[:, vidx]),
        "bp": stage_b(g["proj_b"]),
        "gnw": stage_b(g["gn_w"]),
        "gnb": stage_b(g["gn_b"]),
        "initw": np.ascontiguousarray(
            g["init_w"].T.reshape(SPEC, CC, 128)).astype(bf),
        "initb": np.ascontiguousarray(g["init_b"].reshape(CC, 128).T),
        "ind": np.equal(np.arange(128)[:, None] // 32,
                        np.arange(4)[None, :]).astype(np.float32),
        "indt": np.equal(np.arange(128)[None, :] // 32,
                         np.arange(4)[:, None]).astype(np.float32),
        "ident": np.eye(128, dtype=np.float32).astype(bf),
        "vtinit": _vtinit(bf),
        "sel2": sel2.astype(bf),
        "epsc": np.full((128, 1), EPS, np.float32),
    }
    in_maps = []
    for b in range(B):
        m = dict(common)
        m["speech"] = np.ascontiguousarray(g["speech"][b]).astype(bf)
        in_maps.append(m)
    return in_maps


def kernel(**inputs):
    global LAST_RESULT
    if "nc" not in _CACHE:
        _CACHE["nc"] = _build()
    nc = _CACHE["nc"]
    in_maps = _prep(inputs)
    res = run_bass_kernel_spmd(nc, in_maps, list(range(NCORES)))
    LAST_RESULT = res
    out = np.stack([res.results[b]["out"] for b in range(B)])
    return out.astype(np.float32)


# revision 59
# speedup vs baseline: 1.0023x; 1.0023x over previous
"""Trainium2 Bass kernel for nn_ConditioningEncoder (6-layer attention encoder).

Strategy: data-parallel over batch (B=8 -> 1 batch element per NeuronCore).
All big matmuls run in bf16; f32 accumulation in PSUM throughout.

Per-core computation (C=1024 channels, L=1024 positions, 16 heads, dh=64):
  x = init_w @ speech + init_b                        [C, L] f32
  6x attention blocks:
    h = GroupNorm32(x) * gn_w + gn_b                  (bf16, batched stats)
    k, v = projections of h (head-major channel order)
    vT = PE-transpose of v per head pair + ones column (denominator trick)
    per head-pair p (keeps PE full-width / HAM warm):
      q-projection for chunk p (full 128x128 matmuls)
      chains (t-block): S^T for BOTH heads of the pair issued back-to-back
        as concurrent row-tiled matmuls (tile_position (0,0)/(64,0)) into a
        2-bank PSUM tile; ONE exp ACTIVATE (N=1024) -> et (bf16)
      pav[65,2,t] = [v;1]^T E accumulated over s-chunks (PE), per head
      av_out: DVE copies attn + den rows; den DMA -> [16, L] tile
      pass-2 per pair: rec=1/den (DVE), broadcast via K=2 sel matmul,
        recb copy (DVE), attn *= recb (gpsimd)
    x += proj_w @ attn + proj_b
  out = x[:, 0]

Layer 5 computes q/attention/proj only for t=0..16 (only column 0 returned),
with all 16 S matmuls of a pair batched into one PSUM tile + one exp.
"""
import sys

sys.path.insert(0, "/opt/trn_rl_repo")

from contextlib import ExitStack

import numpy as np
import ml_dtypes

import os

import concourse.bass as bass
import concourse.tile as tile
from concourse import bacc, mybir
from concourse.bass_utils import run_bass_kernel_spmd
from concourse.tile import add_dep_helper

# NOTE: S matmuls with lhsT/rhs at partition base 64 (auto tile_position
# (64,0)) abort on HW when issued adjacent to (0,0) matmuls — instead k is
# staged as two zero-padded copies so every S matmul is a full 128x128
# (K=128 with 64 zero rows; same streaming cost, no tiling modes).
K_SPLIT_EXP = bool(int(os.environ.get("K_SPLIT_EXP", "0")))
K_SPLIT_DMA = bool(int(os.environ.get("K_SPLIT_DMA", "0")))
K_NO_PAIRS = bool(int(os.environ.get("K_NO_PAIRS", "0")))
K_L5_OLD = bool(int(os.environ.get("K_L5_OLD", "0")))

f32 = mybir.dt.float32
f32r = mybir.dt.float32r
bf16 = mybir.dt.bfloat16
AF = mybir.ActivationFunctionType
Alu = mybir.AluOpType

B, SPEC, L = 8, 80, 1024
C, H, DH, NL, NG = 1024, 16, 64, 6, 32
CC = C // 128          # channel chunks per full width
EPS = 1e-5
NCORES = 8

LAST_RESULT = None     # test harness reads exec_time from here
_CACHE = {}


def _build():
    nc = bacc.Bacc("TRN2", target_bir_lowering=False, debug=False,
                   num_devices=NCORES)

    dr = {}
    def din(name, shape, dt):
        dr[name] = nc.dram_tensor(name, shape, dt, kind="ExternalInput").ap()

    din("speech", [SPEC, L], bf16)
    din("initw", [SPEC, CC, 128], bf16)
    din("initb", [128, CC], f32)
    for w in ("wq", "wk", "wv", "wp"):
        din(w, [NL, CC, 128, CC, 128], bf16)
    for b in ("bq", "bk", "bv", "bp", "gnw", "gnb"):
        din(b, [128, NL, CC], f32)
    din("ind", [128, 4], f32r)
    din("indt", [4, 128], f32r)
    din("ident", [128, 128], bf16)
    din("vtinit", [128, CC, 8, 2, 65], bf16)
    din("sel2", [2, 128], bf16)
    din("epsc", [128, 1], f32)
    out_d = nc.dram_tensor("out", [C], f32, kind="ExternalOutput").ap()

    with tile.TileContext(nc) as tc, ExitStack() as ctx:
        cst = ctx.enter_context(tc.tile_pool(name="cst", bufs=1))
        big = ctx.enter_context(tc.tile_pool(name="big", bufs=1))
        wsp = ctx.enter_context(tc.tile_pool(name="wsp", bufs=2))
        ep = ctx.enter_context(tc.tile_pool(name="ep", bufs=2))
        sml = ctx.enter_context(tc.tile_pool(name="sml", bufs=2))
        ps = ctx.enter_context(tc.tile_pool(name="ps", bufs=1, space="PSUM"))

        # ---- constants ----
        ind = cst.tile([128, 4], f32r)
        nc.sync.dma_start(out=ind, in_=dr["ind"])
        indt = cst.tile([4, 128], f32r)
        nc.sync.dma_start(out=indt, in_=dr["indt"])
        ident = cst.tile([128, 128], bf16)
        nc.sync.dma_start(out=ident, in_=dr["ident"])
        sel2 = cst.tile([2, 128], bf16)
        nc.sync.dma_start(out=sel2, in_=dr["sel2"])
        biases = {}
        for b in ("bq", "bk", "bv", "bp", "gnw", "gnb"):
            t = cst.tile([128, NL, CC], f32, name=f"c_{b}")
            nc.sync.dma_start(out=t, in_=dr[b])
            biases[b] = t
        epsc = cst.tile([128, 1], f32)
        nc.sync.dma_start(out=epsc, in_=dr["epsc"])
        initb = cst.tile([128, CC], f32)
        nc.sync.dma_start(out=initb, in_=dr["initb"])

        # persistent activations
        x = big.tile([128, CC, L], f32)
        # vT[s, sc, pair, half, 0:64] = v chans; [..., 64] = 1.0 (denom col)
        vT = big.tile([128, CC, 8, 2, 65], bf16)
        nc.sync.dma_start(out=vT, in_=dr["vtinit"])
        # k staged as two zero-padded copies: kzA rows 64:128 and kzB rows
        # 0:64 stay zero forever, so S matmuls contract over a full K=128.
        kzA = big.tile([128, CC, L], bf16, name="kzA")
        kzB = big.tile([128, CC, L], bf16, name="kzB")
        nc.vector.memset(kzA[64:128, :, :], 0.0)
        nc.vector.memset(kzB[0:64, :, :], 0.0)

        # ---- init conv1x1 ----
        spt = cst.tile([SPEC, L], bf16, name="spt")
        nc.sync.dma_start(out=spt, in_=dr["speech"])
        iwt = cst.tile([SPEC, CC, 128], bf16, name="iwt")
        nc.sync.dma_start(out=iwt, in_=dr["initw"])
        for mc in range(CC):
            pm = ps.tile([128, 2, 512], f32, tag="s", bufs=2, name=f"pi{mc}")
            for tp in range(2):
                nc.tensor.matmul(pm[:, tp, :], lhsT=iwt[:, mc, :],
                                 rhs=spt[:, tp * 512:(tp + 1) * 512],
                                 start=True, stop=True)
            for tp in range(2):
                nc.vector.tensor_scalar(
                    x[:, mc, tp * 512:(tp + 1) * 512], pm[:, tp, :],
                    initb[:, mc:mc + 1], None, Alu.add)

        # ---- layers ----
        for l in range(NL):
            last = (l == NL - 1)
            TW = 16 if last else L     # t-width for q/attn/proj
            TB = 16 if last else 512   # attention t-block

            # GroupNorm: x -> h (bf16), stats batched across chunks
            h = big.tile([128, CC, L], bf16, tag="h", name=f"h{l}")
            st_all = sml.tile([128, CC, 2, 6], f32, tag="bst", bufs=1,
                              name=f"st{l}")
            for cc in range(CC):
                for u in range(2):
                    nc.vector.bn_stats(st_all[:, cc, u, :],
                                       x[:, cc, u * 512:(u + 1) * 512])
            mv = sml.tile([128, CC, 2], f32, tag="mv", bufs=1, name=f"mv{l}")
            for cc in range(CC):
                nc.vector.bn_aggr(mv[:, cc, :], st_all[:, cc, :, :])
            # grhs[:, cc, :] = [mean, var + mean^2]  (f32r)
            grhs = sml.tile([128, CC, 2], f32r, tag="grhs", bufs=1,
                            name=f"gr{l}")
            nc.vector.tensor_copy(out=grhs[:, :, 0], in_=mv[:, :, 0])
            sq = sml.tile([128, CC], f32, tag="sq", bufs=1, name=f"sq{l}")
            nc.vector.tensor_tensor(sq, mv[:, :, 0], mv[:, :, 0], Alu.mult)
            nc.vector.tensor_tensor(grhs[:, :, 1], mv[:, :, 1], sq, Alu.add)
            pg = ps.tile([4, CC, 2], f32, tag="s", bufs=2, name=f"pg{l}")
            nc.tensor.matmul(pg, lhsT=ind, rhs=grhs, start=True, stop=True)
            # group stats -> gmrhs [4, {mean, rstd}, CC] (f32r)
            gmrhs = sml.tile([4, 2, CC], f32r, tag="gm", bufs=1,
                             name=f"gm{l}")
            nc.vector.tensor_scalar(gmrhs[:, 0, :], pg[:, :, 0],
                                    1.0 / 32, None, Alu.mult)
            ex2 = sml.tile([4, CC], f32, tag="ex2", bufs=1, name=f"ex{l}")
            nc.vector.tensor_scalar(ex2, pg[:, :, 1], 1.0 / 32, None,
                                    Alu.mult)
            m2 = sml.tile([4, CC], f32, tag="m2", bufs=1, name=f"m2{l}")
            nc.vector.tensor_tensor(m2, gmrhs[:, 0, :], gmrhs[:, 0, :],
                                    Alu.mult)
            var = sml.tile([4, CC], f32, tag="var", bufs=1, name=f"va{l}")
            nc.vector.tensor_tensor(var, ex2, m2, Alu.subtract)
            sd = sml.tile([4, CC], f32, tag="sd", bufs=1, name=f"sd{l}")
            nc.scalar.activation(sd, var, AF.Sqrt, bias=epsc[0:4, :])
            with nc.allow_low_precision(reason="f32r rstd"):
                nc.vector.reciprocal(gmrhs[:, 1, :], sd)
            pb = ps.tile([128, 2, CC], f32, tag="s", bufs=2, name=f"pb{l}")
            nc.tensor.matmul(pb, lhsT=indt, rhs=gmrhs, start=True, stop=True)
            # scale = rstd*gnw ; shift = gnb - mean*scale
            sc_all = sml.tile([128, CC], f32, tag="scl", bufs=1,
                              name=f"sc{l}")
            nc.vector.tensor_tensor(sc_all, pb[:, 1, :],
                                    biases["gnw"][:, l, :], Alu.mult)
            ms = sml.tile([128, CC], f32, tag="ms", bufs=1, name=f"ms{l}")
            nc.vector.tensor_tensor(ms, pb[:, 0, :], sc_all, Alu.mult)
            sh_all = sml.tile([128, CC], f32, tag="sh", bufs=1, name=f"sh{l}")
            nc.vector.tensor_tensor(sh_all, biases["gnb"][:, l, :], ms,
                                    Alu.subtract)
            for cc in range(CC):
                # gpsimd: DVE is saturated with stats/evacs at layer start
                nc.gpsimd.tensor_scalar(h[:, cc, :], x[:, cc, :],
                                        sc_all[:, cc:cc + 1],
                                        sh_all[:, cc:cc + 1],
                                        Alu.mult, Alu.add)

            # k / v projections (v lands in the attn tile; vT extracted below)
            attn = big.tile([128, CC, L], bf16, tag="avb", name=f"av{l}")
            v = attn
            for dst, w, bias in ((None, "wk", "bk"), (v, "wv", "bv")):
                for mc in range(CC):
                    ws = wsp.tile([128, CC, 128], bf16, tag="ws",
                                  name=f"w{l}_{w}_{mc}")
                    nc.sync.dma_start(out=ws, in_=dr[w][l, mc])
                    pm = ps.tile([128, 2, 512], f32, tag="s", bufs=2,
                                 name=f"p{l}_{w}_{mc}")
                    for tp in range(2):
                        for kc in range(CC):
                            nc.tensor.matmul(pm[:, tp, :], lhsT=ws[:, kc, :],
                                             rhs=h[:, kc,
                                                   tp * 512:(tp + 1) * 512],
                                             start=(kc == 0),
                                             stop=(kc == CC - 1))
                    for tp in range(2):
                        tsl = slice(tp * 512, (tp + 1) * 512)
                        # evacuate on ACT: it idles during the k/v phase
                        # while DVE is saturated with stats/copies
                        if dst is None:   # k: split halves into kzA / kzB
                            nc.scalar.activation(
                                kzA[0:64, mc, tsl], pm[0:64, tp, :],
                                AF.Identity,
                                bias=biases[bias][0:64, l, mc:mc + 1])
                            nc.scalar.activation(
                                kzB[64:128, mc, tsl], pm[64:128, tp, :],
                                AF.Identity,
                                bias=biases[bias][64:128, l, mc:mc + 1])
                        else:
                            # v evacs stay on DVE: ACT must be free when the
                            # attention exps start right after this phase
                            nc.vector.tensor_scalar(
                                dst[:, mc, tsl], pm[:, tp, :],
                                biases[bias][:, l, mc:mc + 1], None, Alu.add)

            # vT: transpose v per head pair; 2 s-chunks per PSUM tile, DVE evac
            for p8 in range(8):
                for sc2 in range(4):
                    pt = ps.tile([128, 2, 2, 64], bf16, tag="s", bufs=2,
                                 name=f"pt{l}_{p8}_{sc2}")
                    for j in range(2):
                        sc = sc2 * 2 + j
                        nc.tensor.transpose(
                            pt[:, j, :, :],
                            v[:, p8, sc * 128:(sc + 1) * 128], ident)
                    nc.vector.tensor_copy(
                        out=vT[:, sc2 * 2:sc2 * 2 + 2, p8, :, 0:64], in_=pt)

            # attention: per head-pair q-projection + S/exp/AV chains
            q = big.tile([128, CC, TW], bf16, tag="q", name=f"q{l}")
            den2 = {}   # pair -> [2, TW] denominator tile (partition base 0)

            def av_mm_one(p8, t0, et, pav, sc, a):
                nc.tensor.matmul(pav[:, a, :],
                                 lhsT=vT[:, sc, p8, a, :],
                                 rhs=et[:, sc, a, :],
                                 start=(sc == 0), stop=(sc == CC - 1),
                                 skip_group_check=True)

            def av_mm(p8, t0, et, pav, sc):
                for a in range(2):
                    av_mm_one(p8, t0, et, pav, sc, a)

            def av_out(p8, t0, pav):
                nc.vector.tensor_copy(
                    out=attn[0:64, p8, t0:t0 + TB], in_=pav[0:64, 0, :])
                nc.vector.tensor_copy(
                    out=attn[64:128, p8, t0:t0 + TB], in_=pav[0:64, 1, :])
                stg = sml.tile([65, 2, TB], f32, tag="stg", bufs=2,
                               name=f"sg{l}_{p8}_{t0}")
                nc.vector.tensor_copy(out=stg[64:65, :, :],
                                      in_=pav[64:65, :, :])
                nc.sync.dma_start(out=den2[p8][:, t0:t0 + TB],
                                  in_=stg[64:65, :, :])

            def pass2(p8):
                # fast approx reciprocal (~18 bits; result is bf16'd anyway)
                r32 = sml.tile([2, TW], f32, tag="r32", bufs=2,
                               name=f"r32{l}_{p8}")
                nc.vector.reciprocal_approx_fast(out=r32, in_=den2[p8])
                rec = sml.tile([2, TW], bf16, tag="rec", bufs=2,
                               name=f"rc{l}_{p8}")
                nc.vector.tensor_copy(out=rec, in_=r32)
                for t0 in range(0, TW, TB):
                    pbc = ps.tile([128, TB], f32, tag="avp", bufs=2,
                                  name=f"pbc{l}_{p8}_{t0}")
                    nc.tensor.matmul(pbc, lhsT=sel2,
                                     rhs=rec[:, t0:t0 + TB],
                                     start=True, stop=True)
                    recb = sml.tile([128, TB], bf16, tag="recb", bufs=2,
                                    name=f"rb{l}_{p8}_{t0}")
                    nc.vector.tensor_copy(out=recb, in_=pbc)
                    nc.gpsimd.tensor_tensor(
                        attn[:, p8, t0:t0 + TB],
                        attn[:, p8, t0:t0 + TB],
                        recb, Alu.mult)

            prev = None
            for p8 in range(8):
                # q projection for this pair's chunk (full-array matmuls)
                wq = wsp.tile([128, CC, 128], bf16, tag="ws",
                              name=f"w{l}_q_{p8}")
                nc.sync.dma_start(out=wq, in_=dr["wq"][l, p8])
                pmq = ps.tile([128, 2, 512] if TW > 16 else [128, 2, 16],
                              f32, tag="s", bufs=2, name=f"pq{l}_{p8}")
                for tp in range(TW // 512 if TW > 16 else 1):
                    tw = min(512, TW)
                    for kc in range(CC):
                        nc.tensor.matmul(
                            pmq[:, tp, :], lhsT=wq[:, kc, :],
                            rhs=h[:, kc, tp * 512:tp * 512 + tw],
                            start=(kc == 0), stop=(kc == CC - 1))
                    nc.vector.tensor_scalar(
                        q[:, p8, tp * 512:tp * 512 + tw], pmq[:, tp, :],
                        biases["bq"][:, l, p8:p8 + 1], None, Alu.add)

                for t0 in range(0, TW, TB):
                    if t0 == 0:
                        den2[p8] = sml.tile([2, TW], f32, tag="den2",
                                            bufs=2, name=f"dn{l}_{p8}")
                    et = ep.tile([128, CC, 2, TB], bf16, tag="E", bufs=2,
                                 name=f"e{l}_{p8}_{t0}")
                    pav = (ps.tile([65, 2, TB], f32, tag="avp", bufs=2,
                                   name=f"pa{l}_{prev[0]}_{prev[1]}")
                           if prev is not None else None)
                    def s_lhs_rhs(a, sc):
                        kz = kzA if a == 0 else kzB
                        return (kz[:, p8, sc * 128:(sc + 1) * 128],
                                q[:, p8, t0:t0 + TB])

                    if TB == 16 and not K_L5_OLD:
                        # layer 5: all 16 S matmuls -> one PSUM tile, one exp
                        pss = ps.tile([128, CC, 2, TB], f32, tag="s", bufs=2,
                                      name=f"ps{l}_{p8}_{t0}")
                        for sc in range(CC):
                            for a in range(2):
                                lh, rh = s_lhs_rhs(a, sc)
                                nc.tensor.matmul(
                                    pss[:, sc, a, :], lhsT=lh, rhs=rh,
                                    start=True, stop=True)
                                if K_NO_PAIRS and prev is not None:
                                    av_mm_one(prev[0], prev[1], prev[2],
                                              pav, sc, a)
                            if not K_NO_PAIRS and prev is not None:
                                av_mm(prev[0], prev[1], prev[2], pav, sc)
                        nc.scalar.activation(et, pss, AF.Exp,
                                             bias=0.0, scale=0.125)
                    else:
                        for sc in range(CC):
                            pss = ps.tile([128, 2, TB], f32, tag="s", bufs=2,
                                          name=f"ps{l}_{p8}_{t0}_{sc}")
                            for a in range(2):
                                lh, rh = s_lhs_rhs(a, sc)
                                nc.tensor.matmul(
                                    pss[:, a, :], lhsT=lh, rhs=rh,
                                    start=True, stop=True)
                                if K_NO_PAIRS and prev is not None:
                                    av_mm_one(prev[0], prev[1], prev[2],
                                              pav, sc, a)
                            if K_SPLIT_EXP:
                                for a in range(2):
                                    nc.scalar.activation(
                                        et[:, sc, a, :], pss[:, a, :],
                                        AF.Exp, bias=0.0, scale=0.125)
                            else:
                                nc.scalar.activation(et[:, sc, :, :], pss,
                                                     AF.Exp, bias=0.0,
                                                     scale=0.125)
                            if not K_NO_PAIRS and prev is not None:
                                av_mm(prev[0], prev[1], prev[2], pav, sc)
                    if prev is not None:
                        av_out(prev[0], prev[1], pav)
                        if prev[1] + TB >= TW:
                            pass2(prev[0])
                    prev = (p8, t0, et)
            pav = ps.tile([65, 2, TB], f32, tag="avp", bufs=2,
                          name=f"pa{l}_{prev[0]}_{prev[1]}")
            for sc in range(CC):
                av_mm(prev[0], prev[1], prev[2], pav, sc)
            av_out(prev[0], prev[1], pav)
            pass2(prev[0])

            # proj + residual
            for mc in range(CC):
                ws = wsp.tile([128, CC, 128], bf16, tag="ws",
                              name=f"wp{l}_{mc}")
                nc.sync.dma_start(out=ws, in_=dr["wp"][l, mc])
                pm = ps.tile([128, 2, 512] if TW > 16 else [128, 2, 16],
                             f32, tag="s", bufs=2, name=f"pp{l}_{mc}")
                for t0 in range(0, TW, 512):
                    tw = min(512, TW - t0)
                    tp = t0 // 512
                    for kc in range(CC):
                        nc.tensor.matmul(pm[:, tp, :], lhsT=ws[:, kc, :],
                                         rhs=attn[:, kc, t0:t0 + tw],
                                         start=(kc == 0), stop=(kc == CC - 1))
                    nc.vector.scalar_tensor_tensor(
                        out=x[:, mc, t0:t0 + tw], in0=pm[:, tp, :],
                        scalar=biases["bp"][:, l, mc:mc + 1],
                        in1=x[:, mc, t0:t0 + tw], op0=Alu.add, op1=Alu.add)

        # ---- output: x[:, :, 0] ----
        o = cst.tile([128, CC], f32)
        nc.vector.tensor_copy(out=o, in_=x[:, :, 0:1].squeeze(-1))
        nc.sync.dma_start(out=out_d.rearrange("(c p) -> p c", p=128), in_=o)

    nc.compile()
    return nc


def _vtinit(bf):
    a = np.zeros((128, CC, 8, 2, 65), np.float32)
    a[:, :, :, :, 64] = 1.0
    return a.astype(bf)


def _prep(inputs):
    """Host-side weight restaging -> per-core input maps."""
    g = {k: np.asarray(v, np.float32) for k, v in inputs.items()}
    bf = ml_dtypes.bfloat16

    idx = np.arange(3 * C).reshape(H, 3, DH)
    qidx, kidx, vidx = idx[:, 0].ravel(), idx[:, 1].ravel(), idx[:, 2].ravel()

    def stage_w(w):            # w [NL, 1024(out), 1024(in)] -> staged lhsT
        wt = w.transpose(0, 2, 1)                    # [l, in, out]
        return np.ascontiguousarray(
            wt.reshape(NL, CC, 128, CC, 128).transpose(0, 3, 2, 1, 4)
        ).astype(bf)

    def stage_b(b):            # [NL, 1024] -> [128, NL, CC]
        return np.ascontiguousarray(
            b.reshape(NL, CC, 128).transpose(2, 0, 1))

    sel2 = np.zeros((2, 128), np.float32)
    for p in range(2):
        sel2[p, p * 64:(p + 1) * 64] = 1.0

    qkv_w, qkv_b = g["qkv_w"], g["qkv_b"]
    common = {
        "wq": stage_w(qkv_w[:, qidx, :]),
        "wk": stage_w(qkv_w[:, kidx, :]),
        "wv": stage_w(qkv_w[:, vidx, :]),
        "wp": stage_w(g["proj_w"]),
        "bq": stage_b(qkv_b[:, qidx]),
        "bk": stage_b(qkv_b[:, kidx]),
        "bv": stage_b(qkv_b[:, vidx]),
        "bp": stage_b(g["proj_b"]),
        "gnw": stage_b(g["gn_w"]),
        "gnb": stage_b(g["gn_b"]),
        "initw": np.ascontiguousarray(
            g["init_w"].T.reshape(SPEC, CC, 128)).astype(bf),
        "initb": np.ascontiguousarray(g["init_b"].reshape(CC, 128).T),
        "ind": np.equal(np.arange(128)[:, None] // 32,
                        np.arange(4)[None, :]).astype(np.float32),
        "indt": np.equal(np.arange(128)[None, :] // 32,
                         np.arange(4)[:, None]).astype(np.float32),
        "ident": np.eye(128, dtype=np.float32).astype(bf),
        "vtinit": _vtinit(bf),
        "sel2": sel2.astype(bf),
        "epsc": np.full((128, 1), EPS, np.float32),
    }
    in_maps = []
    for b in range(B):
        m = dict(common)
        m["speech"] = np.ascontiguousarray(g["speech"][b]).astype(bf)
        in_maps.append(m)
    return in_maps


def kernel(**inputs):
    global LAST_RESULT
    if "nc" not in _CACHE:
        _CACHE["nc"] = _build()
    nc = _CACHE["nc"]
    in_maps = _prep(inputs)
    res = run_bass_kernel_spmd(nc, in_maps, list(range(NCORES)))
    LAST_RESULT = res
    out = np.stack([res.results[b]["out"] for b in range(B)])
    return out.astype(np.float32)


# revision 61
# speedup vs baseline: 1.0164x; 1.0140x over previous
"""Trainium2 Bass kernel for nn_ConditioningEncoder (6-layer attention encoder).

Strategy: data-parallel over batch (B=8 -> 1 batch element per NeuronCore).
All big matmuls run in bf16; f32 accumulation in PSUM throughout.

Per-core computation (C=1024 channels, L=1024 positions, 16 heads, dh=64):
  x = init_w @ speech + init_b                        [C, L] f32
  6x attention blocks:
    h = GroupNorm32(x) * gn_w + gn_b                  (bf16, batched stats)
    k, v = projections of h (head-major channel order)
    vT = PE-transpose of v per head pair + ones column (denominator trick)
    per head-pair p (keeps PE full-width / HAM warm):
      q-projection for chunk p (full 128x128 matmuls)
      chains (t-block): S^T for BOTH heads of the pair issued back-to-back
        as concurrent row-tiled matmuls (tile_position (0,0)/(64,0)) into a
        2-bank PSUM tile; ONE exp ACTIVATE (N=1024) -> et (bf16)
      pav[65,2,t] = [v;1]^T E accumulated over s-chunks (PE), per head
      av_out: DVE copies attn + den rows; den DMA -> [16, L] tile
      pass-2 per pair: rec=1/den (DVE), broadcast via K=2 sel matmul,
        recb copy (DVE), attn *= recb (gpsimd)
    x += proj_w @ attn + proj_b
  out = x[:, 0]

Layer 5 computes q/attention/proj only for t=0..16 (only column 0 returned),
with all 16 S matmuls of a pair batched into one PSUM tile + one exp.
"""
import sys

sys.path.insert(0, "/opt/trn_rl_repo")

from contextlib import ExitStack

import numpy as np
import ml_dtypes

import os

import concourse.bass as bass
import concourse.tile as tile
from concourse import bacc, mybir
from concourse.bass_utils import run_bass_kernel_spmd
from concourse.tile import add_dep_helper

# NOTE: S matmuls with lhsT/rhs at partition base 64 (auto tile_position
# (64,0)) abort on HW when issued adjacent to (0,0) matmuls — instead k is
# staged as two zero-padded copies so every S matmul is a full 128x128
# (K=128 with 64 zero rows; same streaming cost, no tiling modes).
K_SPLIT_EXP = bool(int(os.environ.get("K_SPLIT_EXP", "0")))
K_SPLIT_DMA = bool(int(os.environ.get("K_SPLIT_DMA", "0")))
K_NO_PAIRS = bool(int(os.environ.get("K_NO_PAIRS", "0")))
K_L5_OLD = bool(int(os.environ.get("K_L5_OLD", "0")))

f32 = mybir.dt.float32
f32r = mybir.dt.float32r
bf16 = mybir.dt.bfloat16
AF = mybir.ActivationFunctionType
Alu = mybir.AluOpType

B, SPEC, L = 8, 80, 1024
C, H, DH, NL, NG = 1024, 16, 64, 6, 32
CC = C // 128          # channel chunks per full width
EPS = 1e-5
NCORES = 8

LAST_RESULT = None     # test harness reads exec_time from here
_CACHE = {}


def _build():
    nc = bacc.Bacc("TRN2", target_bir_lowering=False, debug=False,
                   num_devices=NCORES)

    dr = {}
    def din(name, shape, dt):
        dr[name] = nc.dram_tensor(name, shape, dt, kind="ExternalInput").ap()

    din("speech", [SPEC, L], bf16)
    din("initw", [SPEC, CC, 128], bf16)
    din("initb", [128, CC], f32)
    for w in ("wq", "wk", "wv", "wp"):
        din(w, [NL, CC, 128, CC, 128], bf16)
    for b in ("bq", "bk", "bv", "bp", "gnw", "gnb"):
        din(b, [128, NL, CC], f32)
    din("ind", [128, 4], f32r)
    din("indt", [4, 128], f32r)
    din("ident", [128, 128], bf16)
    din("vtinit", [128, CC, 8, 2, 65], bf16)
    din("sel2", [2, 128], bf16)
    din("epsc", [128, 1], f32)
    out_d = nc.dram_tensor("out", [C], f32, kind="ExternalOutput").ap()

    with tile.TileContext(nc) as tc, ExitStack() as ctx:
        cst = ctx.enter_context(tc.tile_pool(name="cst", bufs=1))
        big = ctx.enter_context(tc.tile_pool(name="big", bufs=1))
        wsp = ctx.enter_context(tc.tile_pool(name="wsp", bufs=2))
        ep = ctx.enter_context(tc.tile_pool(name="ep", bufs=2))
        sml = ctx.enter_context(tc.tile_pool(name="sml", bufs=2))
        ps = ctx.enter_context(tc.tile_pool(name="ps", bufs=1, space="PSUM"))

        # ---- constants ----
        ind = cst.tile([128, 4], f32r)
        nc.sync.dma_start(out=ind, in_=dr["ind"])
        indt = cst.tile([4, 128], f32r)
        nc.sync.dma_start(out=indt, in_=dr["indt"])
        ident = cst.tile([128, 128], bf16)
        nc.sync.dma_start(out=ident, in_=dr["ident"])
        sel2 = cst.tile([2, 128], bf16)
        nc.sync.dma_start(out=sel2, in_=dr["sel2"])
        biases = {}
        for b in ("bq", "bk", "bv", "bp", "gnw", "gnb"):
            t = cst.tile([128, NL, CC], f32, name=f"c_{b}")
            nc.sync.dma_start(out=t, in_=dr[b])
            biases[b] = t
        epsc = cst.tile([128, 1], f32)
        nc.sync.dma_start(out=epsc, in_=dr["epsc"])
        initb = cst.tile([128, CC], f32)
        nc.sync.dma_start(out=initb, in_=dr["initb"])

        # persistent activations
        x = big.tile([128, CC, L], f32)
        # vT[s, sc, pair, half, 0:64] = v chans; [..., 64] = 1.0 (denom col)
        vT = big.tile([128, CC, 8, 2, 65], bf16)
        nc.sync.dma_start(out=vT, in_=dr["vtinit"])
        # k staged as two zero-padded copies: kzA rows 64:128 and kzB rows
        # 0:64 stay zero forever, so S matmuls contract over a full K=128.
        kzA = big.tile([128, CC, L], bf16, name="kzA")
        kzB = big.tile([128, CC, L], bf16, name="kzB")
        nc.vector.memset(kzA[64:128, :, :], 0.0)
        nc.vector.memset(kzB[0:64, :, :], 0.0)

        # ---- init conv1x1 ----
        spt = cst.tile([SPEC, L], bf16, name="spt")
        nc.sync.dma_start(out=spt, in_=dr["speech"])
        iwt = cst.tile([SPEC, CC, 128], bf16, name="iwt")
        nc.sync.dma_start(out=iwt, in_=dr["initw"])
        for mc in range(CC):
            pm = ps.tile([128, 2, 512], f32, tag="s", bufs=2, name=f"pi{mc}")
            for tp in range(2):
                nc.tensor.matmul(pm[:, tp, :], lhsT=iwt[:, mc, :],
                                 rhs=spt[:, tp * 512:(tp + 1) * 512],
                                 start=True, stop=True)
            for tp in range(2):
                nc.vector.tensor_scalar(
                    x[:, mc, tp * 512:(tp + 1) * 512], pm[:, tp, :],
                    initb[:, mc:mc + 1], None, Alu.add)

        # ---- layers ----
        for l in range(NL):
            last = (l == NL - 1)
            TW = 16 if last else L     # t-width for q/attn/proj
            TB = 16 if last else 512   # attention t-block

            # GroupNorm: x -> h (bf16), stats batched across chunks
            h = big.tile([128, CC, L], bf16, tag="h", name=f"h{l}")
            st_all = sml.tile([128, CC, 2, 6], f32, tag="bst", bufs=1,
                              name=f"st{l}")
            for cc in range(CC):
                for u in range(2):
                    nc.vector.bn_stats(st_all[:, cc, u, :],
                                       x[:, cc, u * 512:(u + 1) * 512])
            mv = sml.tile([128, CC, 2], f32, tag="mv", bufs=1, name=f"mv{l}")
            for cc in range(CC):
                nc.vector.bn_aggr(mv[:, cc, :], st_all[:, cc, :, :])
            # grhs[:, cc, :] = [mean, var + mean^2]  (f32r)
            grhs = sml.tile([128, CC, 2], f32r, tag="grhs", bufs=1,
                            name=f"gr{l}")
            nc.vector.tensor_copy(out=grhs[:, :, 0], in_=mv[:, :, 0])
            sq = sml.tile([128, CC], f32, tag="sq", bufs=1, name=f"sq{l}")
            nc.vector.tensor_tensor(sq, mv[:, :, 0], mv[:, :, 0], Alu.mult)
            nc.vector.tensor_tensor(grhs[:, :, 1], mv[:, :, 1], sq, Alu.add)
            pg = ps.tile([4, CC, 2], f32, tag="s", bufs=2, name=f"pg{l}")
            nc.tensor.matmul(pg, lhsT=ind, rhs=grhs, start=True, stop=True)
            # group stats -> gmrhs [4, {mean, rstd}, CC] (f32r)
            gmrhs = sml.tile([4, 2, CC], f32r, tag="gm", bufs=1,
                             name=f"gm{l}")
            nc.vector.tensor_scalar(gmrhs[:, 0, :], pg[:, :, 0],
                                    1.0 / 32, None, Alu.mult)
            ex2 = sml.tile([4, CC], f32, tag="ex2", bufs=1, name=f"ex{l}")
            nc.vector.tensor_scalar(ex2, pg[:, :, 1], 1.0 / 32, None,
                                    Alu.mult)
            m2 = sml.tile([4, CC], f32, tag="m2", bufs=1, name=f"m2{l}")
            nc.vector.tensor_tensor(m2, gmrhs[:, 0, :], gmrhs[:, 0, :],
                                    Alu.mult)
            var = sml.tile([4, CC], f32, tag="var", bufs=1, name=f"va{l}")
            nc.vector.scalar_tensor_tensor(
                out=var, in0=ex2, scalar=EPS, in1=m2,
                op0=Alu.add, op1=Alu.subtract)  # var = (ex2+eps) - m2
            # rstd via DVE-only Newton rsqrt: keeps ACT pinned on the exp
            # table set (avoids 2 ACT table loads per layer)
            yi = sml.tile([4, CC], mybir.dt.int32, tag="yi", bufs=1,
                          name=f"yi{l}")
            yi2 = sml.tile([4, CC], mybir.dt.int32, tag="yi2", bufs=1,
                           name=f"yj{l}")
            nc.vector.tensor_scalar(yi, var.bitcast(mybir.dt.int32),
                                    1, None, Alu.logical_shift_right)
            nc.vector.tensor_scalar(yi2, yi, -1, 0x5F3759DF,
                                    Alu.mult, Alu.add)
            y = yi2.bitcast(f32)
            yn = sml.tile([4, CC], f32, tag="yn", bufs=1, name=f"yn{l}")
            tq = sml.tile([4, CC], f32, tag="tq", bufs=1, name=f"tq{l}")
            for it in range(2):
                nc.vector.tensor_tensor(tq, y, y, Alu.mult)
                nc.vector.tensor_tensor(tq, tq, var, Alu.mult)
                nc.vector.tensor_scalar(tq, tq, -0.5, 1.5,
                                        Alu.mult, Alu.add)
                dst = yn if it == 0 else gmrhs[:, 1, :]
                if it == 0:
                    nc.vector.tensor_tensor(dst, y, tq, Alu.mult)
                else:
                    with nc.allow_low_precision(reason="f32r rstd"):
                        nc.vector.tensor_tensor(dst, y, tq, Alu.mult)
                y = yn
            pb = ps.tile([128, 2, CC], f32, tag="s", bufs=2, name=f"pb{l}")
            nc.tensor.matmul(pb, lhsT=indt, rhs=gmrhs, start=True, stop=True)
            # scale = rstd*gnw ; shift = gnb - mean*scale
            sc_all = sml.tile([128, CC], f32, tag="scl", bufs=1,
                              name=f"sc{l}")
            nc.vector.tensor_tensor(sc_all, pb[:, 1, :],
                                    biases["gnw"][:, l, :], Alu.mult)
            ms = sml.tile([128, CC], f32, tag="ms", bufs=1, name=f"ms{l}")
            nc.vector.tensor_tensor(ms, pb[:, 0, :], sc_all, Alu.mult)
            sh_all = sml.tile([128, CC], f32, tag="sh", bufs=1, name=f"sh{l}")
            nc.vector.tensor_tensor(sh_all, biases["gnb"][:, l, :], ms,
                                    Alu.subtract)
            for cc in range(CC):
                # gpsimd: DVE is saturated with stats/evacs at layer start
                nc.gpsimd.tensor_scalar(h[:, cc, :], x[:, cc, :],
                                        sc_all[:, cc:cc + 1],
                                        sh_all[:, cc:cc + 1],
                                        Alu.mult, Alu.add)

            # k / v projections (v lands in the attn tile; vT extracted below)
            attn = big.tile([128, CC, L], bf16, tag="avb", name=f"av{l}")
            v = attn
            for dst, w, bias in ((None, "wk", "bk"), (v, "wv", "bv")):
                for mc in range(CC):
                    ws = wsp.tile([128, CC, 128], bf16, tag="ws",
                                  name=f"w{l}_{w}_{mc}")
                    nc.sync.dma_start(out=ws, in_=dr[w][l, mc])
                    pm = ps.tile([128, 2, 512], f32, tag="s", bufs=2,
                                 name=f"p{l}_{w}_{mc}")
                    for tp in range(2):
                        for kc in range(CC):
                            nc.tensor.matmul(pm[:, tp, :], lhsT=ws[:, kc, :],
                                             rhs=h[:, kc,
                                                   tp * 512:(tp + 1) * 512],
                                             start=(kc == 0),
                                             stop=(kc == CC - 1))
                    for tp in range(2):
                        tsl = slice(tp * 512, (tp + 1) * 512)
                        # evacuate on ACT: it idles during the k/v phase
                        # while DVE is saturated with stats/copies
                        if dst is None:   # k: split halves into kzA / kzB
                            nc.scalar.activation(
                                kzA[0:64, mc, tsl], pm[0:64, tp, :],
                                AF.Identity,
                                bias=biases[bias][0:64, l, mc:mc + 1])
                            nc.scalar.activation(
                                kzB[64:128, mc, tsl], pm[64:128, tp, :],
                                AF.Identity,
                                bias=biases[bias][64:128, l, mc:mc + 1])
                        else:
                            # v evacs stay on DVE: ACT must be free when the
                            # attention exps start right after this phase
                            nc.vector.tensor_scalar(
                                dst[:, mc, tsl], pm[:, tp, :],
                                biases[bias][:, l, mc:mc + 1], None, Alu.add)

            # vT: transpose v per head pair; 2 s-chunks per PSUM tile, DVE evac
            for p8 in range(8):
                for sc2 in range(4):
                    pt = ps.tile([128, 2, 2, 64], bf16, tag="s", bufs=2,
                                 name=f"pt{l}_{p8}_{sc2}")
                    for j in range(2):
                        sc = sc2 * 2 + j
                        nc.tensor.transpose(
                            pt[:, j, :, :],
                            v[:, p8, sc * 128:(sc + 1) * 128], ident)
                    nc.vector.tensor_copy(
                        out=vT[:, sc2 * 2:sc2 * 2 + 2, p8, :, 0:64], in_=pt)

            # attention: per head-pair q-projection + S/exp/AV chains
            q = big.tile([128, CC, TW], bf16, tag="q", name=f"q{l}")
            den2 = {}   # pair -> [2, TW] denominator tile (partition base 0)

            def av_mm_one(p8, t0, et, pav, sc, a):
                nc.tensor.matmul(pav[:, a, :],
                                 lhsT=vT[:, sc, p8, a, :],
                                 rhs=et[:, sc, a, :],
                                 start=(sc == 0), stop=(sc == CC - 1),
                                 skip_group_check=True)

            def av_mm(p8, t0, et, pav, sc):
                for a in range(2):
                    av_mm_one(p8, t0, et, pav, sc, a)

            def av_out(p8, t0, pav):
                nc.vector.tensor_copy(
                    out=attn[0:64, p8, t0:t0 + TB], in_=pav[0:64, 0, :])
                nc.vector.tensor_copy(
                    out=attn[64:128, p8, t0:t0 + TB], in_=pav[0:64, 1, :])
                stg = sml.tile([65, 2, TB], f32, tag="stg", bufs=2,
                               name=f"sg{l}_{p8}_{t0}")
                nc.vector.tensor_copy(out=stg[64:65, :, :],
                                      in_=pav[64:65, :, :])
                nc.sync.dma_start(out=den2[p8][:, t0:t0 + TB],
                                  in_=stg[64:65, :, :])

            def pass2(p8):
                # fast approx reciprocal (~18 bits; result is bf16'd anyway)
                r32 = sml.tile([2, TW], f32, tag="r32", bufs=2,
                               name=f"r32{l}_{p8}")
                nc.vector.reciprocal_approx_fast(out=r32, in_=den2[p8])
                rec = sml.tile([2, TW], bf16, tag="rec", bufs=2,
                               name=f"rc{l}_{p8}")
                nc.vector.tensor_copy(out=rec, in_=r32)
                for t0 in range(0, TW, TB):
                    pbc = ps.tile([128, TB], f32, tag="avp", bufs=2,
                                  name=f"pbc{l}_{p8}_{t0}")
                    nc.tensor.matmul(pbc, lhsT=sel2,
                                     rhs=rec[:, t0:t0 + TB],
                                     start=True, stop=True)
                    recb = sml.tile([128, TB], bf16, tag="recb", bufs=2,
                                    name=f"rb{l}_{p8}_{t0}")
                    nc.vector.tensor_copy(out=recb, in_=pbc)
                    nc.gpsimd.tensor_tensor(
                        attn[:, p8, t0:t0 + TB],
                        attn[:, p8, t0:t0 + TB],
                        recb, Alu.mult)

            prev = None
            for p8 in range(8):
                # q projection for this pair's chunk (full-array matmuls)
                wq = wsp.tile([128, CC, 128], bf16, tag="ws",
                              name=f"w{l}_q_{p8}")
                nc.sync.dma_start(out=wq, in_=dr["wq"][l, p8])
                pmq = ps.tile([128, 2, 512] if TW > 16 else [128, 2, 16],
                              f32, tag="s", bufs=2, name=f"pq{l}_{p8}")
                for tp in range(TW // 512 if TW > 16 else 1):
                    tw = min(512, TW)
                    for kc in range(CC):
                        nc.tensor.matmul(
                            pmq[:, tp, :], lhsT=wq[:, kc, :],
                            rhs=h[:, kc, tp * 512:tp * 512 + tw],
                            start=(kc == 0), stop=(kc == CC - 1))
                    nc.vector.tensor_scalar(
                        q[:, p8, tp * 512:tp * 512 + tw], pmq[:, tp, :],
                        biases["bq"][:, l, p8:p8 + 1], None, Alu.add)

                for t0 in range(0, TW, TB):
                    if t0 == 0:
                        den2[p8] = sml.tile([2, TW], f32, tag="den2",
                                            bufs=2, name=f"dn{l}_{p8}")
                    et = ep.tile([128, CC, 2, TB], bf16, tag="E", bufs=2,
                                 name=f"e{l}_{p8}_{t0}")
                    pav = (ps.tile([65, 2, TB], f32, tag="avp", bufs=2,
                                   name=f"pa{l}_{prev[0]}_{prev[1]}")
                           if prev is not None else None)
                    def s_lhs_rhs(a, sc):
                        kz = kzA if a == 0 else kzB
                        return (kz[:, p8, sc * 128:(sc + 1) * 128],
                                q[:, p8, t0:t0 + TB])

                    if TB == 16 and not K_L5_OLD:
                        # layer 5: all 16 S matmuls -> one PSUM tile, one exp
                        pss = ps.tile([128, CC, 2, TB], f32, tag="s", bufs=2,
                                      name=f"ps{l}_{p8}_{t0}")
                        for sc in range(CC):
                            for a in range(2):
                                lh, rh = s_lhs_rhs(a, sc)
                                nc.tensor.matmul(
                                    pss[:, sc, a, :], lhsT=lh, rhs=rh,
                                    start=True, stop=True)
                                if K_NO_PAIRS and prev is not None:
                                    av_mm_one(prev[0], prev[1], prev[2],
                                              pav, sc, a)
                            if not K_NO_PAIRS and prev is not None:
                                av_mm(prev[0], prev[1], prev[2], pav, sc)
                        nc.scalar.activation(et, pss, AF.Exp,
                                             bias=0.0, scale=0.125)
                    else:
                        for sc in range(CC):
                            pss = ps.tile([128, 2, TB], f32, tag="s", bufs=2,
                                          name=f"ps{l}_{p8}_{t0}_{sc}")
                            for a in range(2):
                                lh, rh = s_lhs_rhs(a, sc)
                                nc.tensor.matmul(
                                    pss[:, a, :], lhsT=lh, rhs=rh,
                                    start=True, stop=True)
                                if K_NO_PAIRS and prev is not None:
                                    av_mm_one(prev[0], prev[1], prev[2],
                                              pav, sc, a)
                            if K_SPLIT_EXP:
                                for a in range(2):
                                    nc.scalar.activation(
                                        et[:, sc, a, :], pss[:, a, :],
                                        AF.Exp, bias=0.0, scale=0.125)
                            else:
                                nc.scalar.activation(et[:, sc, :, :], pss,
                                                     AF.Exp, bias=0.0,
                                                     scale=0.125)
                            if not K_NO_PAIRS and prev is not None:
                                av_mm(prev[0], prev[1], prev[2], pav, sc)
                    if prev is not None:
                        av_out(prev[0], prev[1], pav)
                        if prev[1] + TB >= TW:
                            pass2(prev[0])
                    prev = (p8, t0, et)
            pav = ps.tile([65, 2, TB], f32, tag="avp", bufs=2,
                          name=f"pa{l}_{prev[0]}_{prev[1]}")
            for sc in range(CC):
                av_mm(prev[0], prev[1], prev[2], pav, sc)
            av_out(prev[0], prev[1], pav)
            pass2(prev[0])

            # proj + residual
            for mc in range(CC):
                ws = wsp.tile([128, CC, 128], bf16, tag="ws",
                              name=f"wp{l}_{mc}")
                nc.sync.dma_start(out=ws, in_=dr["wp"][l, mc])
                pm = ps.tile([128, 2, 512] if TW > 16 else [128, 2, 16],
                             f32, tag="s", bufs=2, name=f"pp{l}_{mc}")
                for t0 in range(0, TW, 512):
                    tw = min(512, TW - t0)
                    tp = t0 // 512
                    for kc in range(CC):
                        nc.tensor.matmul(pm[:, tp, :], lhsT=ws[:, kc, :],
                                         rhs=attn[:, kc, t0:t0 + tw],
                                         start=(kc == 0), stop=(kc == CC - 1))
                    nc.vector.scalar_tensor_tensor(
                        out=x[:, mc, t0:t0 + tw], in0=pm[:, tp, :],
                        scalar=biases["bp"][:, l, mc:mc + 1],
                        in1=x[:, mc, t0:t0 + tw], op0=Alu.add, op1=Alu.add)

        # ---- output: x[:, :, 0] ----
        o = cst.tile([128, CC], f32)
        nc.vector.tensor_copy(out=o, in_=x[:, :, 0:1].squeeze(-1))
        nc.sync.dma_start(out=out_d.rearrange("(c p) -> p c", p=128), in_=o)

    nc.compile()
    return nc


def _vtinit(bf):
    a = np.zeros((128, CC, 8, 2, 65), np.float32)
    a[:, :, :, :, 64] = 1.0
    return a.astype(bf)


def _prep(inputs):
    """Host-side weight restaging -> per-core input maps."""
    g = {k: np.asarray(v, np.float32) for k, v in inputs.items()}
    bf = ml_dtypes.bfloat16

    idx = np.arange(3 * C).reshape(H, 3, DH)
    qidx, kidx, vidx = idx[:, 0].ravel(), idx[:, 1].ravel(), idx[:, 2].ravel()

    def stage_w(w):            # w [NL, 1024(out), 1024(in)] -> staged lhsT
        wt = w.transpose(0, 2, 1)                    # [l, in, out]
        return np.ascontiguousarray(
            wt.reshape(NL, CC, 128, CC, 128).transpose(0, 3, 2, 1, 4)
        ).astype(bf)

    def stage_b(b):            # [NL, 1024] -> [128, NL, CC]
        return np.ascontiguousarray(
            b.reshape(NL, CC, 128).transpose(2, 0, 1))

    sel2 = np.zeros((2, 128), np.float32)
    for p in range(2):
        sel2[p, p * 64:(p + 1) * 64] = 1.0

    qkv_w, qkv_b = g["qkv_w"], g["qkv_b"]
    common = {
        "wq": stage_w(qkv_w[:, qidx, :]),
        "wk": stage_w(qkv_w[:, kidx, :]),
        "wv": stage_w(qkv_w[:, vidx, :]),
        "wp": stage_w(g["proj_w"]),
        "bq": stage_b(qkv_b[:, qidx]),
        "bk": stage_b(qkv_b[:, kidx]),
        "bv": stage_b(qkv_b[:, vidx]),
        "bp": stage_b(g["proj_b"]),
        "gnw": stage_b(g["gn_w"]),
        "gnb": stage_b(g["gn_b"]),
        "initw": np.ascontiguousarray(
            g["init_w"].T.reshape(SPEC, CC, 128)).astype(bf),
        "initb": np.ascontiguousarray(g["init_b"].reshape(CC, 128).T),
        "ind": np.equal(np.arange(128)[:, None] // 32,
                        np.arange(4)[None, :]).astype(np.float32),
        "indt": np.equal(np.arange(128)[None, :] // 32,
                         np.arange(4)[:, None]).astype(np.float32),
        "ident": np.eye(128, dtype=np.float32).astype(bf),
        "vtinit": _vtinit(bf),
        "sel2": sel2.astype(bf),
        "epsc": np.full((128, 1), EPS, np.float32),
    }
    in_maps = []
    for b in range(B):
        m = dict(common)
        m["speech"] = np.ascontiguousarray(g["speech"][b]).astype(bf)
        in_maps.append(m)
    return in_maps


def kernel(**inputs):
    global LAST_RESULT
    if "nc" not in _CACHE:
        _CACHE["nc"] = _build()
    nc = _CACHE["nc"]
    in_maps = _prep(inputs)
    res = run_bass_kernel_spmd(nc, in_maps, list(range(NCORES)))
    LAST_RESULT = res
    out = np.stack([res.results[b]["out"] for b in range(B)])
    return out.astype(np.float32)


# revision 64
# speedup vs baseline: 1.0476x; 1.0307x over previous
"""Trainium2 Bass kernel for nn_ConditioningEncoder (6-layer attention encoder).

Strategy: data-parallel over batch (B=8 -> 1 batch element per NeuronCore).
All big matmuls run in bf16; f32 accumulation in PSUM throughout.

Per-core computation (C=1024 channels, L=1024 positions, 16 heads, dh=64):
  x = init_w @ speech + init_b                        [C, L] f32
  6x attention blocks:
    h = GroupNorm32(x) * gn_w + gn_b                  (bf16, batched stats)
    k, v = projections of h (head-major channel order)
    vT = PE-transpose of v per head pair + ones column (denominator trick)
    per head-pair p (keeps PE full-width / HAM warm):
      q-projection for chunk p (full 128x128 matmuls)
      chains (t-block): S^T for BOTH heads of the pair issued back-to-back
        as concurrent row-tiled matmuls (tile_position (0,0)/(64,0)) into a
        2-bank PSUM tile; ONE exp ACTIVATE (N=1024) -> et (bf16)
      pav[65,2,t] = [v;1]^T E accumulated over s-chunks (PE), per head
      av_out: DVE copies attn + den rows; den DMA -> [16, L] tile
      pass-2 per pair: rec=1/den (DVE), broadcast via K=2 sel matmul,
        recb copy (DVE), attn *= recb (gpsimd)
    x += proj_w @ attn + proj_b
  out = x[:, 0]

Layer 5 computes q/attention/proj only for t=0..16 (only column 0 returned),
with all 16 S matmuls of a pair batched into one PSUM tile + one exp.
"""
import sys

sys.path.insert(0, "/opt/trn_rl_repo")

from contextlib import ExitStack

import numpy as np
import ml_dtypes

import os

import concourse.bass as bass
import concourse.tile as tile
from concourse import bacc, mybir
from concourse.bass_utils import run_bass_kernel_spmd
from concourse.tile import add_dep_helper

# NOTE: S matmuls with lhsT/rhs at partition base 64 (auto tile_position
# (64,0)) abort on HW when issued adjacent to (0,0) matmuls — instead k is
# staged as two zero-padded copies so every S matmul is a full 128x128
# (K=128 with 64 zero rows; same streaming cost, no tiling modes).
K_SPLIT_EXP = bool(int(os.environ.get("K_SPLIT_EXP", "0")))
K_SPLIT_DMA = bool(int(os.environ.get("K_SPLIT_DMA", "0")))
K_NO_PAIRS = bool(int(os.environ.get("K_NO_PAIRS", "0")))
K_L5_OLD = bool(int(os.environ.get("K_L5_OLD", "0")))

f32 = mybir.dt.float32
f32r = mybir.dt.float32r
bf16 = mybir.dt.bfloat16
AF = mybir.ActivationFunctionType
Alu = mybir.AluOpType

B, SPEC, L = 8, 80, 1024
C, H, DH, NL, NG = 1024, 16, 64, 6, 32
CC = C // 128          # channel chunks per full width
EPS = 1e-5
NCORES = 8

LAST_RESULT = None     # test harness reads exec_time from here
_CACHE = {}


def _build():
    nc = bacc.Bacc("TRN2", target_bir_lowering=False, debug=False,
                   num_devices=NCORES)

    dr = {}
    def din(name, shape, dt):
        dr[name] = nc.dram_tensor(name, shape, dt, kind="ExternalInput").ap()

    din("speech", [SPEC, L], bf16)
    din("initw", [SPEC, CC, 128], bf16)
    din("initb", [128, CC], f32)
    for w in ("wq", "wk", "wv", "wp"):
        din(w, [NL, CC, 128, CC, 128], bf16)
    for b in ("bq", "bk", "bv", "bp", "gnw", "gnb"):
        din(b, [128, NL, CC], f32)
    din("ind", [128, 4], f32r)
    din("indt", [4, 128], f32r)
    din("ident", [128, 128], bf16)
    din("vtinit", [128, CC, 8, 2, 65], bf16)
    din("sel2", [2, 128], bf16)
    din("epsc", [128, 1], f32)
    out_d = nc.dram_tensor("out", [C], f32, kind="ExternalOutput").ap()

    with tile.TileContext(nc) as tc, ExitStack() as ctx:
        cst = ctx.enter_context(tc.tile_pool(name="cst", bufs=1))
        big = ctx.enter_context(tc.tile_pool(name="big", bufs=1))
        wsp = ctx.enter_context(tc.tile_pool(name="wsp", bufs=2))
        ep = ctx.enter_context(tc.tile_pool(name="ep", bufs=2))
        sml = ctx.enter_context(tc.tile_pool(name="sml", bufs=2))
        ps = ctx.enter_context(tc.tile_pool(name="ps", bufs=1, space="PSUM"))

        # ---- constants ----
        ind = cst.tile([128, 4], f32r)
        nc.sync.dma_start(out=ind, in_=dr["ind"])
        indt = cst.tile([4, 128], f32r)
        nc.sync.dma_start(out=indt, in_=dr["indt"])
        ident = cst.tile([128, 128], bf16)
        nc.sync.dma_start(out=ident, in_=dr["ident"])
        sel2 = cst.tile([2, 128], bf16)
        nc.sync.dma_start(out=sel2, in_=dr["sel2"])
        biases = {}
        for b in ("bq", "bk", "bv", "bp", "gnw", "gnb"):
            t = cst.tile([128, NL, CC], f32, name=f"c_{b}")
            nc.sync.dma_start(out=t, in_=dr[b])
            biases[b] = t
        epsc = cst.tile([128, 1], f32)
        nc.sync.dma_start(out=epsc, in_=dr["epsc"])
        initb = cst.tile([128, CC], f32)
        nc.sync.dma_start(out=initb, in_=dr["initb"])

        # persistent activations
        x = big.tile([128, CC, L], f32)
        # vT[s, sc, pair, half, 0:64] = v chans; [..., 64] = 1.0 (denom col)
        vT = big.tile([128, CC, 8, 2, 65], bf16)
        nc.sync.dma_start(out=vT, in_=dr["vtinit"])
        # k staged as two zero-padded copies: kzA rows 64:128 and kzB rows
        # 0:64 stay zero forever, so S matmuls contract over a full K=128.
        kzA = big.tile([128, CC, L], bf16, name="kzA")
        kzB = big.tile([128, CC, L], bf16, name="kzB")
        nc.vector.memset(kzA[64:128, :, :], 0.0)
        nc.vector.memset(kzB[0:64, :, :], 0.0)

        # ---- init conv1x1 ----
        spt = cst.tile([SPEC, L], bf16, name="spt")
        nc.sync.dma_start(out=spt, in_=dr["speech"])
        iwt = cst.tile([SPEC, CC, 128], bf16, name="iwt")
        nc.sync.dma_start(out=iwt, in_=dr["initw"])
        for mc in range(CC):
            pm = ps.tile([128, 2, 512], f32, tag="s", bufs=2, name=f"pi{mc}")
            for tp in range(2):
                nc.tensor.matmul(pm[:, tp, :], lhsT=iwt[:, mc, :],
                                 rhs=spt[:, tp * 512:(tp + 1) * 512],
                                 start=True, stop=True)
            for tp in range(2):
                nc.vector.tensor_scalar(
                    x[:, mc, tp * 512:(tp + 1) * 512], pm[:, tp, :],
                    initb[:, mc:mc + 1], None, Alu.add)

        # ---- layers ----
        for l in range(NL):
            last = (l == NL - 1)
            TW = 16 if last else L     # t-width for q/attn/proj
            TB = 16 if last else 512   # attention t-block

            # GroupNorm: x -> h (bf16), stats batched across chunks
            h = big.tile([128, CC, L], bf16, tag="h", name=f"h{l}")
            st_all = sml.tile([128, CC, 2, 6], f32, tag="bst", bufs=1,
                              name=f"st{l}")
            for cc in range(CC):
                for u in range(2):
                    nc.vector.bn_stats(st_all[:, cc, u, :],
                                       x[:, cc, u * 512:(u + 1) * 512])
            mv = sml.tile([128, CC, 2], f32, tag="mv", bufs=1, name=f"mv{l}")
            for cc in range(CC):
                nc.vector.bn_aggr(mv[:, cc, :], st_all[:, cc, :, :])
            # grhs[:, cc, :] = [mean, var + mean^2]  (f32r)
            grhs = sml.tile([128, CC, 2], f32r, tag="grhs", bufs=1,
                            name=f"gr{l}")
            nc.vector.tensor_copy(out=grhs[:, :, 0], in_=mv[:, :, 0])
            sq = sml.tile([128, CC], f32, tag="sq", bufs=1, name=f"sq{l}")
            nc.vector.tensor_tensor(sq, mv[:, :, 0], mv[:, :, 0], Alu.mult)
            nc.vector.tensor_tensor(grhs[:, :, 1], mv[:, :, 1], sq, Alu.add)
            pg = ps.tile([4, CC, 2], f32, tag="s", bufs=2, name=f"pg{l}")
            nc.tensor.matmul(pg, lhsT=ind, rhs=grhs, start=True, stop=True)
            # group stats -> gmrhs [4, {mean, rstd}, CC] (f32r)
            gmrhs = sml.tile([4, 2, CC], f32r, tag="gm", bufs=1,
                             name=f"gm{l}")
            nc.vector.tensor_scalar(gmrhs[:, 0, :], pg[:, :, 0],
                                    1.0 / 32, None, Alu.mult)
            ex2 = sml.tile([4, CC], f32, tag="ex2", bufs=1, name=f"ex{l}")
            nc.vector.tensor_scalar(ex2, pg[:, :, 1], 1.0 / 32, None,
                                    Alu.mult)
            m2 = sml.tile([4, CC], f32, tag="m2", bufs=1, name=f"m2{l}")
            nc.vector.tensor_tensor(m2, gmrhs[:, 0, :], gmrhs[:, 0, :],
                                    Alu.mult)
            var = sml.tile([4, CC], f32, tag="var", bufs=1, name=f"va{l}")
            nc.vector.scalar_tensor_tensor(
                out=var, in0=ex2, scalar=EPS, in1=m2,
                op0=Alu.add, op1=Alu.subtract)  # var = (ex2+eps) - m2
            # rstd via DVE-only Newton rsqrt: keeps ACT pinned on the exp
            # table set (avoids 2 ACT table loads per layer)
            yi = sml.tile([4, CC], mybir.dt.int32, tag="yi", bufs=1,
                          name=f"yi{l}")
            yi2 = sml.tile([4, CC], mybir.dt.int32, tag="yi2", bufs=1,
                           name=f"yj{l}")
            nc.vector.tensor_scalar(yi, var.bitcast(mybir.dt.int32),
                                    1, None, Alu.logical_shift_right)
            nc.vector.tensor_scalar(yi2, yi, -1, 0x5F3759DF,
                                    Alu.mult, Alu.add)
            y = yi2.bitcast(f32)
            yn = sml.tile([4, CC], f32, tag="yn", bufs=1, name=f"yn{l}")
            tq = sml.tile([4, CC], f32, tag="tq", bufs=1, name=f"tq{l}")
            for it in range(2):
                nc.vector.tensor_tensor(tq, y, y, Alu.mult)
                nc.vector.tensor_tensor(tq, tq, var, Alu.mult)
                nc.vector.tensor_scalar(tq, tq, -0.5, 1.5,
                                        Alu.mult, Alu.add)
                dst = yn if it == 0 else gmrhs[:, 1, :]
                if it == 0:
                    nc.vector.tensor_tensor(dst, y, tq, Alu.mult)
                else:
                    with nc.allow_low_precision(reason="f32r rstd"):
                        nc.vector.tensor_tensor(dst, y, tq, Alu.mult)
                y = yn
            pb = ps.tile([128, 2, CC], f32, tag="s", bufs=2, name=f"pb{l}")
            nc.tensor.matmul(pb, lhsT=indt, rhs=gmrhs, start=True, stop=True)
            # scale = rstd*gnw ; shift = gnb - mean*scale
            sc_all = sml.tile([128, CC], f32, tag="scl", bufs=1,
                              name=f"sc{l}")
            nc.vector.tensor_tensor(sc_all, pb[:, 1, :],
                                    biases["gnw"][:, l, :], Alu.mult)
            ms = sml.tile([128, CC], f32, tag="ms", bufs=1, name=f"ms{l}")
            nc.vector.tensor_tensor(ms, pb[:, 0, :], sc_all, Alu.mult)
            sh_all = sml.tile([128, CC], f32, tag="sh", bufs=1, name=f"sh{l}")
            nc.vector.tensor_tensor(sh_all, biases["gnb"][:, l, :], ms,
                                    Alu.subtract)
            for cc in range(CC):
                # split across gpsimd/DVE to halve the GN-tail latency
                eng = nc.gpsimd if cc % 2 == 0 else nc.vector
                eng.tensor_scalar(h[:, cc, :], x[:, cc, :],
                                  sc_all[:, cc:cc + 1],
                                  sh_all[:, cc:cc + 1],
                                  Alu.mult, Alu.add)

            # k / v projections (v lands in the attn tile; vT extracted below)
            attn = big.tile([128, CC, L], bf16, tag="avb", name=f"av{l}")
            v = attn
            for dst, w, bias in ((None, "wk", "bk"), (v, "wv", "bv")):
                for mc in range(CC):
                    ws = wsp.tile([128, CC, 128], bf16, tag="ws",
                                  name=f"w{l}_{w}_{mc}")
                    nc.sync.dma_start(out=ws, in_=dr[w][l, mc])
                    pm = ps.tile([128, 2, 512], f32, tag="s", bufs=2,
                                 name=f"p{l}_{w}_{mc}")
                    for tp in range(2):
                        for kc in range(CC):
                            nc.tensor.matmul(pm[:, tp, :], lhsT=ws[:, kc, :],
                                             rhs=h[:, kc,
                                                   tp * 512:(tp + 1) * 512],
                                             start=(kc == 0),
                                             stop=(kc == CC - 1))
                    for tp in range(2):
                        tsl = slice(tp * 512, (tp + 1) * 512)
                        # evacuate on ACT: it idles during the k/v phase
                        # while DVE is saturated with stats/copies
                        if dst is None:   # k: split halves into kzA / kzB
                            nc.scalar.activation(
                                kzA[0:64, mc, tsl], pm[0:64, tp, :],
                                AF.Identity,
                                bias=biases[bias][0:64, l, mc:mc + 1])
                            nc.scalar.activation(
                                kzB[64:128, mc, tsl], pm[64:128, tp, :],
                                AF.Identity,
                                bias=biases[bias][64:128, l, mc:mc + 1])
                        else:
                            # v evacs stay on DVE: ACT must be free when the
                            # attention exps start right after this phase
                            nc.vector.tensor_scalar(
                                dst[:, mc, tsl], pm[:, tp, :],
                                biases[bias][:, l, mc:mc + 1], None, Alu.add)

            # attention: per head-pair q-projection + S/exp/AV chains
            q = big.tile([128, CC, TW], bf16, tag="q", name=f"q{l}")
            den2 = {}   # pair -> [2, TW] denominator tile (partition base 0)

            def av_mm_one(p8, t0, et, pav, sc, a):
                nc.tensor.matmul(pav[:, a, :],
                                 lhsT=vT[:, sc, p8, a, :],
                                 rhs=et[:, sc, a, :],
                                 start=(sc == 0), stop=(sc == CC - 1),
                                 skip_group_check=True)

            def av_mm(p8, t0, et, pav, sc):
                for a in range(2):
                    av_mm_one(p8, t0, et, pav, sc, a)

            def av_out(p8, t0, pav):
                nc.vector.tensor_copy(
                    out=attn[0:64, p8, t0:t0 + TB], in_=pav[0:64, 0, :])
                nc.vector.tensor_copy(
                    out=attn[64:128, p8, t0:t0 + TB], in_=pav[0:64, 1, :])
                stg = sml.tile([65, 2, TB], f32, tag="stg", bufs=2,
                               name=f"sg{l}_{p8}_{t0}")
                nc.vector.tensor_copy(out=stg[64:65, :, :],
                                      in_=pav[64:65, :, :])
                nc.sync.dma_start(out=den2[p8][:, t0:t0 + TB],
                                  in_=stg[64:65, :, :])

            def pass2(p8):
                # fast approx reciprocal (~18 bits; result is bf16'd anyway)
                r32 = sml.tile([2, TW], f32, tag="r32", bufs=2,
                               name=f"r32{l}_{p8}")
                nc.vector.reciprocal_approx_fast(out=r32, in_=den2[p8])
                rec = sml.tile([2, TW], bf16, tag="rec", bufs=2,
                               name=f"rc{l}_{p8}")
                nc.vector.tensor_copy(out=rec, in_=r32)
                for t0 in range(0, TW, TB):
                    pbc = ps.tile([128, TB], f32, tag="avp", bufs=2,
                                  name=f"pbc{l}_{p8}_{t0}")
                    nc.tensor.matmul(pbc, lhsT=sel2,
                                     rhs=rec[:, t0:t0 + TB],
                                     start=True, stop=True)
                    recb = sml.tile([128, TB], bf16, tag="recb", bufs=2,
                                    name=f"rb{l}_{p8}_{t0}")
                    nc.vector.tensor_copy(out=recb, in_=pbc)
                    nc.gpsimd.tensor_tensor(
                        attn[:, p8, t0:t0 + TB],
                        attn[:, p8, t0:t0 + TB],
                        recb, Alu.mult)

            prev = None
            for p8 in range(8):
                # q projection for this pair's chunk (full-array matmuls)
                wq = wsp.tile([128, CC, 128], bf16, tag="ws",
                              name=f"w{l}_q_{p8}")
                nc.sync.dma_start(out=wq, in_=dr["wq"][l, p8])
                pmq = ps.tile([128, 2, 512] if TW > 16 else [128, 2, 16],
                              f32, tag="s", bufs=2, name=f"pq{l}_{p8}")
                for tp in range(TW // 512 if TW > 16 else 1):
                    tw = min(512, TW)
                    for kc in range(CC):
                        nc.tensor.matmul(
                            pmq[:, tp, :], lhsT=wq[:, kc, :],
                            rhs=h[:, kc, tp * 512:tp * 512 + tw],
                            start=(kc == 0), stop=(kc == CC - 1))
                    nc.vector.tensor_scalar(
                        q[:, p8, tp * 512:tp * 512 + tw], pmq[:, tp, :],
                        biases["bq"][:, l, p8:p8 + 1], None, Alu.add)

                # vT for this pair: transposes ride in the ACT-gated PE slack
                for sc2 in range(4):
                    pt = ps.tile([128, 2, 2, 64], bf16, tag="s", bufs=2,
                                 name=f"pt{l}_{p8}_{sc2}")
                    for j in range(2):
                        sc = sc2 * 2 + j
                        nc.tensor.transpose(
                            pt[:, j, :, :],
                            v[:, p8, sc * 128:(sc + 1) * 128], ident)
                    nc.vector.tensor_copy(
                        out=vT[:, sc2 * 2:sc2 * 2 + 2, p8, :, 0:64], in_=pt)

                for t0 in range(0, TW, TB):
                    if t0 == 0:
                        den2[p8] = sml.tile([2, TW], f32, tag="den2",
                                            bufs=2, name=f"dn{l}_{p8}")
                    et = ep.tile([128, CC, 2, TB], bf16, tag="E", bufs=2,
                                 name=f"e{l}_{p8}_{t0}")
                    pav = (ps.tile([65, 2, TB], f32, tag="avp", bufs=2,
                                   name=f"pa{l}_{prev[0]}_{prev[1]}")
                           if prev is not None else None)
                    def s_lhs_rhs(a, sc):
                        kz = kzA if a == 0 else kzB
                        return (kz[:, p8, sc * 128:(sc + 1) * 128],
                                q[:, p8, t0:t0 + TB])

                    if TB == 16 and not K_L5_OLD:
                        # layer 5: all 16 S matmuls -> one PSUM tile, one exp
                        pss = ps.tile([128, CC, 2, TB], f32, tag="s", bufs=2,
                                      name=f"ps{l}_{p8}_{t0}")
                        for sc in range(CC):
                            for a in range(2):
                                lh, rh = s_lhs_rhs(a, sc)
                                nc.tensor.matmul(
                                    pss[:, sc, a, :], lhsT=lh, rhs=rh,
                                    start=True, stop=True)
                                if K_NO_PAIRS and prev is not None:
                                    av_mm_one(prev[0], prev[1], prev[2],
                                              pav, sc, a)
                            if not K_NO_PAIRS and prev is not None:
                                av_mm(prev[0], prev[1], prev[2], pav, sc)
                        nc.scalar.activation(et, pss, AF.Exp,
                                             bias=0.0, scale=0.125)
                    else:
                        for sc in range(CC):
                            pss = ps.tile([128, 2, TB], f32, tag="s", bufs=2,
                                          name=f"ps{l}_{p8}_{t0}_{sc}")
                            for a in range(2):
                                lh, rh = s_lhs_rhs(a, sc)
                                nc.tensor.matmul(
                                    pss[:, a, :], lhsT=lh, rhs=rh,
                                    start=True, stop=True)
                                if K_NO_PAIRS and prev is not None:
                                    av_mm_one(prev[0], prev[1], prev[2],
                                              pav, sc, a)
                            if K_SPLIT_EXP:
                                for a in range(2):
                                    nc.scalar.activation(
                                        et[:, sc, a, :], pss[:, a, :],
                                        AF.Exp, bias=0.0, scale=0.125)
                            else:
                                nc.scalar.activation(et[:, sc, :, :], pss,
                                                     AF.Exp, bias=0.0,
                                                     scale=0.125)
                            if not K_NO_PAIRS and prev is not None:
                                av_mm(prev[0], prev[1], prev[2], pav, sc)
                    if prev is not None:
                        av_out(prev[0], prev[1], pav)
                        if prev[1] + TB >= TW:
                            pass2(prev[0])
                    prev = (p8, t0, et)
            pav = ps.tile([65, 2, TB], f32, tag="avp", bufs=2,
                          name=f"pa{l}_{prev[0]}_{prev[1]}")
            for sc in range(CC):
                av_mm(prev[0], prev[1], prev[2], pav, sc)
            av_out(prev[0], prev[1], pav)
            pass2(prev[0])

            # proj + residual
            for mc in range(CC):
                ws = wsp.tile([128, CC, 128], bf16, tag="ws",
                              name=f"wp{l}_{mc}")
                nc.sync.dma_start(out=ws, in_=dr["wp"][l, mc])
                pm = ps.tile([128, 2, 512] if TW > 16 else [128, 2, 16],
                             f32, tag="s", bufs=2, name=f"pp{l}_{mc}")
                for t0 in range(0, TW, 512):
                    tw = min(512, TW - t0)
                    tp = t0 // 512
                    for kc in range(CC):
                        nc.tensor.matmul(pm[:, tp, :], lhsT=ws[:, kc, :],
                                         rhs=attn[:, kc, t0:t0 + tw],
                                         start=(kc == 0), stop=(kc == CC - 1))
                    nc.vector.scalar_tensor_tensor(
                        out=x[:, mc, t0:t0 + tw], in0=pm[:, tp, :],
                        scalar=biases["bp"][:, l, mc:mc + 1],
                        in1=x[:, mc, t0:t0 + tw], op0=Alu.add, op1=Alu.add)

        # ---- output: x[:, :, 0] ----
        o = cst.tile([128, CC], f32)
        nc.vector.tensor_copy(out=o, in_=x[:, :, 0:1].squeeze(-1))
        nc.sync.dma_start(out=out_d.rearrange("(c p) -> p c", p=128), in_=o)

    nc.compile()
    return nc


def _vtinit(bf):
    a = np.zeros((128, CC, 8, 2, 65), np.float32)
    a[:, :, :, :, 64] = 1.0
    return a.astype(bf)


def _prep(inputs):
    """Host-side weight restaging -> per-core input maps."""
    g = {k: np.asarray(v, np.float32) for k, v in inputs.items()}
    bf = ml_dtypes.bfloat16

    idx = np.arange(3 * C).reshape(H, 3, DH)
    qidx, kidx, vidx = idx[:, 0].ravel(), idx[:, 1].ravel(), idx[:, 2].ravel()

    def stage_w(w):            # w [NL, 1024(out), 1024(in)] -> staged lhsT
        wt = w.transpose(0, 2, 1)                    # [l, in, out]
        return np.ascontiguousarray(
            wt.reshape(NL, CC, 128, CC, 128).transpose(0, 3, 2, 1, 4)
        ).astype(bf)

    def stage_b(b):            # [NL, 1024] -> [128, NL, CC]
        return np.ascontiguousarray(
            b.reshape(NL, CC, 128).transpose(2, 0, 1))

    sel2 = np.zeros((2, 128), np.float32)
    for p in range(2):
        sel2[p, p * 64:(p + 1) * 64] = 1.0

    qkv_w, qkv_b = g["qkv_w"], g["qkv_b"]
    common = {
        "wq": stage_w(qkv_w[:, qidx, :]),
        "wk": stage_w(qkv_w[:, kidx, :]),
        "wv": stage_w(qkv_w[:, vidx, :]),
        "wp": stage_w(g["proj_w"]),
        "bq": stage_b(qkv_b[:, qidx]),
        "bk": stage_b(qkv_b[:, kidx]),
        "bv": stage_b(qkv_b[:, vidx]),
        "bp": stage_b(g["proj_b"]),
        "gnw": stage_b(g["gn_w"]),
        "gnb": stage_b(g["gn_b"]),
        "initw": np.ascontiguousarray(
            g["init_w"].T.reshape(SPEC, CC, 128)).astype(bf),
        "initb": np.ascontiguousarray(g["init_b"].reshape(CC, 128).T),
        "ind": np.equal(np.arange(128)[:, None] // 32,
                        np.arange(4)[None, :]).astype(np.float32),
        "indt": np.equal(np.arange(128)[None, :] // 32,
                         np.arange(4)[:, None]).astype(np.float32),
        "ident": np.eye(128, dtype=np.float32).astype(bf),
        "vtinit": _vtinit(bf),
        "sel2": sel2.astype(bf),
        "epsc": np.full((128, 1), EPS, np.float32),
    }
    in_maps = []
    for b in range(B):
        m = dict(common)
        m["speech"] = np.ascontiguousarray(g["speech"][b]).astype(bf)
        in_maps.append(m)
    return in_maps


def kernel(**inputs):
    global LAST_RESULT
    if "nc" not in _CACHE:
        _CACHE["nc"] = _build()
    nc = _CACHE["nc"]
    in_maps = _prep(inputs)
    res = run_bass_kernel_spmd(nc, in_maps, list(range(NCORES)))
    LAST_RESULT = res
    out = np.stack([res.results[b]["out"] for b in range(B)])
    return out.astype(np.float32)


# revision 66
# speedup vs baseline: 1.0706x; 1.0219x over previous
"""Trainium2 Bass kernel for nn_ConditioningEncoder (6-layer attention encoder).

Strategy: data-parallel over batch (B=8 -> 1 batch element per NeuronCore).
All big matmuls run in bf16; f32 accumulation in PSUM throughout.

Per-core computation (C=1024 channels, L=1024 positions, 16 heads, dh=64):
  x = init_w @ speech + init_b                        [C, L] f32
  6x attention blocks:
    h = GroupNorm32(x) * gn_w + gn_b                  (bf16, batched stats)
    k, v = projections of h (head-major channel order)
    vT = PE-transpose of v per head pair + ones column (denominator trick)
    per head-pair p (keeps PE full-width / HAM warm):
      q-projection for chunk p (full 128x128 matmuls)
      chains (t-block): S^T for BOTH heads of the pair issued back-to-back
        as concurrent row-tiled matmuls (tile_position (0,0)/(64,0)) into a
        2-bank PSUM tile; ONE exp ACTIVATE (N=1024) -> et (bf16)
      pav[65,2,t] = [v;1]^T E accumulated over s-chunks (PE), per head
      av_out: DVE copies attn + den rows; den DMA -> [16, L] tile
      pass-2 per pair: rec=1/den (DVE), broadcast via K=2 sel matmul,
        recb copy (DVE), attn *= recb (gpsimd)
    x += proj_w @ attn + proj_b
  out = x[:, 0]

Layer 5 computes q/attention/proj only for t=0..16 (only column 0 returned),
with all 16 S matmuls of a pair batched into one PSUM tile + one exp.
"""
import sys

sys.path.insert(0, "/opt/trn_rl_repo")

from contextlib import ExitStack

import numpy as np
import ml_dtypes

import os

import concourse.bass as bass
import concourse.tile as tile
from concourse import bacc, mybir
from concourse.bass_utils import run_bass_kernel_spmd
from concourse.tile import add_dep_helper

# NOTE: S matmuls with lhsT/rhs at partition base 64 (auto tile_position
# (64,0)) abort on HW when issued adjacent to (0,0) matmuls — instead k is
# staged as two zero-padded copies so every S matmul is a full 128x128
# (K=128 with 64 zero rows; same streaming cost, no tiling modes).
K_SPLIT_EXP = bool(int(os.environ.get("K_SPLIT_EXP", "0")))
K_SPLIT_DMA = bool(int(os.environ.get("K_SPLIT_DMA", "0")))
K_NO_PAIRS = bool(int(os.environ.get("K_NO_PAIRS", "0")))
K_L5_OLD = bool(int(os.environ.get("K_L5_OLD", "0")))

f32 = mybir.dt.float32
f32r = mybir.dt.float32r
bf16 = mybir.dt.bfloat16
AF = mybir.ActivationFunctionType
Alu = mybir.AluOpType

B, SPEC, L = 8, 80, 1024
C, H, DH, NL, NG = 1024, 16, 64, 6, 32
CC = C // 128          # channel chunks per full width
EPS = 1e-5
NCORES = 8

LAST_RESULT = None     # test harness reads exec_time from here
_CACHE = {}


def _build():
    nc = bacc.Bacc("TRN2", target_bir_lowering=False, debug=False,
                   num_devices=NCORES)

    dr = {}
    def din(name, shape, dt):
        dr[name] = nc.dram_tensor(name, shape, dt, kind="ExternalInput").ap()

    din("speech", [SPEC, L], bf16)
    din("initw", [SPEC, CC, 128], bf16)
    din("initb", [128, CC], f32)
    for w in ("wq", "wk", "wv", "wp"):
        din(w, [NL, CC, 128, CC, 128], bf16)
    for b in ("bq", "bk", "bv", "bp", "gnw", "gnb"):
        din(b, [128, NL, CC], f32)
    din("ind", [128, 4], f32r)
    din("indt", [4, 128], f32r)
    din("ident", [128, 128], bf16)
    din("vtinit", [128, CC, 8, 2, 65], bf16)
    din("sel2", [2, 128], bf16)
    din("epsc", [128, 1], f32)
    out_d = nc.dram_tensor("out", [C], f32, kind="ExternalOutput").ap()

    with tile.TileContext(nc) as tc, ExitStack() as ctx:
        cst = ctx.enter_context(tc.tile_pool(name="cst", bufs=1))
        big = ctx.enter_context(tc.tile_pool(name="big", bufs=1))
        wsp = ctx.enter_context(tc.tile_pool(name="wsp", bufs=2))
        ep = ctx.enter_context(tc.tile_pool(name="ep", bufs=2))
        sml = ctx.enter_context(tc.tile_pool(name="sml", bufs=2))
        ps = ctx.enter_context(tc.tile_pool(name="ps", bufs=1, space="PSUM"))

        # ---- constants ----
        ind = cst.tile([128, 4], f32r)
        nc.sync.dma_start(out=ind, in_=dr["ind"])
        indt = cst.tile([4, 128], f32r)
        nc.sync.dma_start(out=indt, in_=dr["indt"])
        ident = cst.tile([128, 128], bf16)
        nc.sync.dma_start(out=ident, in_=dr["ident"])
        sel2 = cst.tile([2, 128], bf16)
        nc.sync.dma_start(out=sel2, in_=dr["sel2"])
        biases = {}
        for b in ("bq", "bk", "bv", "bp", "gnw", "gnb"):
            t = cst.tile([128, NL, CC], f32, name=f"c_{b}")
            nc.sync.dma_start(out=t, in_=dr[b])
            biases[b] = t
        epsc = cst.tile([128, 1], f32)
        nc.sync.dma_start(out=epsc, in_=dr["epsc"])
        initb = cst.tile([128, CC], f32)
        nc.sync.dma_start(out=initb, in_=dr["initb"])

        # persistent activations
        x = big.tile([128, CC, L], f32)
        # vT[s, sc, pair, half, 0:64] = v chans; [..., 64] = 1.0 (denom col)
        vT = big.tile([128, CC, 8, 2, 65], bf16)
        nc.sync.dma_start(out=vT, in_=dr["vtinit"])
        # k staged as two zero-padded copies: kzA rows 64:128 and kzB rows
        # 0:64 stay zero forever, so S matmuls contract over a full K=128.
        kzA = big.tile([128, CC, L], bf16, name="kzA")
        kzB = big.tile([128, CC, L], bf16, name="kzB")
        nc.vector.memset(kzA[64:128, :, :], 0.0)
        nc.vector.memset(kzB[0:64, :, :], 0.0)

        # ---- init conv1x1 ----
        spt = cst.tile([SPEC, L], bf16, name="spt")
        nc.sync.dma_start(out=spt, in_=dr["speech"])
        iwt = cst.tile([SPEC, CC, 128], bf16, name="iwt")
        nc.sync.dma_start(out=iwt, in_=dr["initw"])
        for mc in range(CC):
            pm = ps.tile([128, 2, 512], f32, tag="s", bufs=2, name=f"pi{mc}")
            for tp in range(2):
                nc.tensor.matmul(pm[:, tp, :], lhsT=iwt[:, mc, :],
                                 rhs=spt[:, tp * 512:(tp + 1) * 512],
                                 start=True, stop=True)
            for tp in range(2):
                nc.vector.tensor_scalar(
                    x[:, mc, tp * 512:(tp + 1) * 512], pm[:, tp, :],
                    initb[:, mc:mc + 1], None, Alu.add)

        # ---- layers ----
        for l in range(NL):
            last = (l == NL - 1)
            TW = 16 if last else L     # t-width for q/attn/proj
            TB = 16 if last else 512   # attention t-block

            # GroupNorm: x -> h (bf16), stats batched across chunks
            h = big.tile([128, CC, L], bf16, tag="h", name=f"h{l}")
            st_all = sml.tile([128, CC, 2, 6], f32, tag="bst", bufs=1,
                              name=f"st{l}")
            for cc in range(CC):
                for u in range(2):
                    nc.vector.bn_stats(st_all[:, cc, u, :],
                                       x[:, cc, u * 512:(u + 1) * 512])
            mv = sml.tile([128, CC, 2], f32, tag="mv", bufs=1, name=f"mv{l}")
            for cc in range(CC):
                nc.vector.bn_aggr(mv[:, cc, :], st_all[:, cc, :, :])
            # grhs[:, cc, :] = [mean, var + mean^2]  (f32r)
            grhs = sml.tile([128, CC, 2], f32r, tag="grhs", bufs=1,
                            name=f"gr{l}")
            nc.vector.tensor_copy(out=grhs[:, :, 0], in_=mv[:, :, 0])
            sq = sml.tile([128, CC], f32, tag="sq", bufs=1, name=f"sq{l}")
            nc.vector.tensor_tensor(sq, mv[:, :, 0], mv[:, :, 0], Alu.mult)
            nc.vector.tensor_tensor(grhs[:, :, 1], mv[:, :, 1], sq, Alu.add)
            pg = ps.tile([4, CC, 2], f32, tag="s", bufs=2, name=f"pg{l}")
            nc.tensor.matmul(pg, lhsT=ind, rhs=grhs, start=True, stop=True)
            # group stats -> gmrhs [4, {mean, rstd}, CC] (f32r)
            gmrhs = sml.tile([4, 2, CC], f32r, tag="gm", bufs=1,
                             name=f"gm{l}")
            nc.vector.tensor_scalar(gmrhs[:, 0, :], pg[:, :, 0],
                                    1.0 / 32, None, Alu.mult)
            ex2 = sml.tile([4, CC], f32, tag="ex2", bufs=1, name=f"ex{l}")
            nc.vector.tensor_scalar(ex2, pg[:, :, 1], 1.0 / 32, None,
                                    Alu.mult)
            m2 = sml.tile([4, CC], f32, tag="m2", bufs=1, name=f"m2{l}")
            nc.vector.tensor_tensor(m2, gmrhs[:, 0, :], gmrhs[:, 0, :],
                                    Alu.mult)
            var = sml.tile([4, CC], f32, tag="var", bufs=1, name=f"va{l}")
            nc.vector.scalar_tensor_tensor(
                out=var, in0=ex2, scalar=EPS, in1=m2,
                op0=Alu.add, op1=Alu.subtract)  # var = (ex2+eps) - m2
            # rstd via DVE-only Newton rsqrt: keeps ACT pinned on the exp
            # table set (avoids 2 ACT table loads per layer)
            yi = sml.tile([4, CC], mybir.dt.int32, tag="yi", bufs=1,
                          name=f"yi{l}")
            yi2 = sml.tile([4, CC], mybir.dt.int32, tag="yi2", bufs=1,
                           name=f"yj{l}")
            nc.vector.tensor_scalar(yi, var.bitcast(mybir.dt.int32),
                                    1, None, Alu.logical_shift_right)
            nc.vector.tensor_scalar(yi2, yi, -1, 0x5F3759DF,
                                    Alu.mult, Alu.add)
            y = yi2.bitcast(f32)
            yn = sml.tile([4, CC], f32, tag="yn", bufs=1, name=f"yn{l}")
            tq = sml.tile([4, CC], f32, tag="tq", bufs=1, name=f"tq{l}")
            for it in range(2):
                nc.vector.tensor_tensor(tq, y, y, Alu.mult)
                nc.vector.tensor_tensor(tq, tq, var, Alu.mult)
                nc.vector.tensor_scalar(tq, tq, -0.5, 1.5,
                                        Alu.mult, Alu.add)
                dst = yn if it == 0 else gmrhs[:, 1, :]
                if it == 0:
                    nc.vector.tensor_tensor(dst, y, tq, Alu.mult)
                else:
                    with nc.allow_low_precision(reason="f32r rstd"):
                        nc.vector.tensor_tensor(dst, y, tq, Alu.mult)
                y = yn
            pb = ps.tile([128, 2, CC], f32, tag="s", bufs=2, name=f"pb{l}")
            nc.tensor.matmul(pb, lhsT=indt, rhs=gmrhs, start=True, stop=True)
            # scale = rstd*gnw ; shift = gnb - mean*scale
            sc_all = sml.tile([128, CC], f32, tag="scl", bufs=1,
                              name=f"sc{l}")
            nc.vector.tensor_tensor(sc_all, pb[:, 1, :],
                                    biases["gnw"][:, l, :], Alu.mult)
            ms = sml.tile([128, CC], f32, tag="ms", bufs=1, name=f"ms{l}")
            nc.vector.tensor_tensor(ms, pb[:, 0, :], sc_all, Alu.mult)
            sh_all = sml.tile([128, CC], f32, tag="sh", bufs=1, name=f"sh{l}")
            nc.vector.tensor_tensor(sh_all, biases["gnb"][:, l, :], ms,
                                    Alu.subtract)
            for cc in range(CC):
                # split across gpsimd/DVE to halve the GN-tail latency
                eng = nc.gpsimd if cc % 2 == 0 else nc.vector
                eng.tensor_scalar(h[:, cc, :], x[:, cc, :],
                                  sc_all[:, cc:cc + 1],
                                  sh_all[:, cc:cc + 1],
                                  Alu.mult, Alu.add)

            # k / v are produced per-pair inside the attention loop: chain
            # (pair p) only reads k-chunk p and v-chunk p, so their matmuls
            # overlap the ACT-gated exp stream of earlier pairs' chains.
            attn = big.tile([128, CC, L], bf16, tag="avb", name=f"av{l}")
            v = attn

            def kv_chunk(p8):
                for dst, w, bias in ((None, "wk", "bk"), (v, "wv", "bv")):
                    ws = wsp.tile([128, CC, 128], bf16, tag="ws",
                                  name=f"w{l}_{w}_{p8}")
                    nc.sync.dma_start(out=ws, in_=dr[w][l, p8])
                    pm = ps.tile([128, 2, 512], f32, tag="s", bufs=2,
                                 name=f"p{l}_{w}_{p8}")
                    for tp in range(2):
                        for kc in range(CC):
                            nc.tensor.matmul(pm[:, tp, :], lhsT=ws[:, kc, :],
                                             rhs=h[:, kc,
                                                   tp * 512:(tp + 1) * 512],
                                             start=(kc == 0),
                                             stop=(kc == CC - 1))
                    for tp in range(2):
                        tsl = slice(tp * 512, (tp + 1) * 512)
                        if dst is None:   # k halves into kzA / kzB, on ACT
                            nc.scalar.activation(
                                kzA[0:64, p8, tsl], pm[0:64, tp, :],
                                AF.Identity,
                                bias=biases[bias][0:64, l, p8:p8 + 1])
                            nc.scalar.activation(
                                kzB[64:128, p8, tsl], pm[64:128, tp, :],
                                AF.Identity,
                                bias=biases[bias][64:128, l, p8:p8 + 1])
                        else:             # v on DVE
                            nc.vector.tensor_scalar(
                                dst[:, p8, tsl], pm[:, tp, :],
                                biases[bias][:, l, p8:p8 + 1], None, Alu.add)

            # attention: per head-pair kv/q projections + S/exp/AV chains
            q = big.tile([128, CC, TW], bf16, tag="q", name=f"q{l}")
            den2 = {}   # pair -> [2, TW] denominator tile (partition base 0)

            def av_mm_one(p8, t0, et, pav, sc, a):
                nc.tensor.matmul(pav[:, a, :],
                                 lhsT=vT[:, sc, p8, a, :],
                                 rhs=et[:, sc, a, :],
                                 start=(sc == 0), stop=(sc == CC - 1),
                                 skip_group_check=True)

            def av_mm(p8, t0, et, pav, sc):
                for a in range(2):
                    av_mm_one(p8, t0, et, pav, sc, a)

            def av_out(p8, t0, pav):
                nc.vector.tensor_copy(
                    out=attn[0:64, p8, t0:t0 + TB], in_=pav[0:64, 0, :])
                nc.vector.tensor_copy(
                    out=attn[64:128, p8, t0:t0 + TB], in_=pav[0:64, 1, :])
                stg = sml.tile([65, 2, TB], f32, tag="stg", bufs=2,
                               name=f"sg{l}_{p8}_{t0}")
                nc.vector.tensor_copy(out=stg[64:65, :, :],
                                      in_=pav[64:65, :, :])
                nc.sync.dma_start(out=den2[p8][:, t0:t0 + TB],
                                  in_=stg[64:65, :, :])

            def pass2(p8):
                # fast approx reciprocal (~18 bits; result is bf16'd anyway)
                r32 = sml.tile([2, TW], f32, tag="r32", bufs=2,
                               name=f"r32{l}_{p8}")
                nc.vector.reciprocal_approx_fast(out=r32, in_=den2[p8])
                rec = sml.tile([2, TW], bf16, tag="rec", bufs=2,
                               name=f"rc{l}_{p8}")
                nc.vector.tensor_copy(out=rec, in_=r32)
                for t0 in range(0, TW, TB):
                    pbc = ps.tile([128, TB], f32, tag="avp", bufs=2,
                                  name=f"pbc{l}_{p8}_{t0}")
                    nc.tensor.matmul(pbc, lhsT=sel2,
                                     rhs=rec[:, t0:t0 + TB],
                                     start=True, stop=True)
                    recb = sml.tile([128, TB], bf16, tag="recb", bufs=2,
                                    name=f"rb{l}_{p8}_{t0}")
                    nc.vector.tensor_copy(out=recb, in_=pbc)
                    nc.gpsimd.tensor_tensor(
                        attn[:, p8, t0:t0 + TB],
                        attn[:, p8, t0:t0 + TB],
                        recb, Alu.mult)

            prev = None
            for p8 in range(8):
                kv_chunk(p8)
                # q projection for this pair's chunk (full-array matmuls)
                wq = wsp.tile([128, CC, 128], bf16, tag="ws",
                              name=f"w{l}_q_{p8}")
                nc.sync.dma_start(out=wq, in_=dr["wq"][l, p8])
                pmq = ps.tile([128, 2, 512] if TW > 16 else [128, 2, 16],
                              f32, tag="s", bufs=2, name=f"pq{l}_{p8}")
                for tp in range(TW // 512 if TW > 16 else 1):
                    tw = min(512, TW)
                    for kc in range(CC):
                        nc.tensor.matmul(
                            pmq[:, tp, :], lhsT=wq[:, kc, :],
                            rhs=h[:, kc, tp * 512:tp * 512 + tw],
                            start=(kc == 0), stop=(kc == CC - 1))
                    nc.vector.tensor_scalar(
                        q[:, p8, tp * 512:tp * 512 + tw], pmq[:, tp, :],
                        biases["bq"][:, l, p8:p8 + 1], None, Alu.add)

                # vT for this pair: transposes ride in the ACT-gated PE slack
                for sc2 in range(4):
                    pt = ps.tile([128, 2, 2, 64], bf16, tag="s", bufs=2,
                                 name=f"pt{l}_{p8}_{sc2}")
                    for j in range(2):
                        sc = sc2 * 2 + j
                        nc.tensor.transpose(
                            pt[:, j, :, :],
                            v[:, p8, sc * 128:(sc + 1) * 128], ident)
                    nc.vector.tensor_copy(
                        out=vT[:, sc2 * 2:sc2 * 2 + 2, p8, :, 0:64], in_=pt)

                for t0 in range(0, TW, TB):
                    if t0 == 0:
                        den2[p8] = sml.tile([2, TW], f32, tag="den2",
                                            bufs=2, name=f"dn{l}_{p8}")
                    et = ep.tile([128, CC, 2, TB], bf16, tag="E", bufs=2,
                                 name=f"e{l}_{p8}_{t0}")
                    pav = (ps.tile([65, 2, TB], f32, tag="avp", bufs=2,
                                   name=f"pa{l}_{prev[0]}_{prev[1]}")
                           if prev is not None else None)
                    def s_lhs_rhs(a, sc):
                        kz = kzA if a == 0 else kzB
                        return (kz[:, p8, sc * 128:(sc + 1) * 128],
                                q[:, p8, t0:t0 + TB])

                    if TB == 16 and not K_L5_OLD:
                        # layer 5: all 16 S matmuls -> one PSUM tile, one exp
                        pss = ps.tile([128, CC, 2, TB], f32, tag="s", bufs=2,
                                      name=f"ps{l}_{p8}_{t0}")
                        for sc in range(CC):
                            for a in range(2):
                                lh, rh = s_lhs_rhs(a, sc)
                                nc.tensor.matmul(
                                    pss[:, sc, a, :], lhsT=lh, rhs=rh,
                                    start=True, stop=True)
                                if K_NO_PAIRS and prev is not None:
                                    av_mm_one(prev[0], prev[1], prev[2],
                                              pav, sc, a)
                            if not K_NO_PAIRS and prev is not None:
                                av_mm(prev[0], prev[1], prev[2], pav, sc)
                        nc.scalar.activation(et, pss, AF.Exp,
                                             bias=0.0, scale=0.125)
                    else:
                        for sc in range(CC):
                            pss = ps.tile([128, 2, TB], f32, tag="s", bufs=2,
                                          name=f"ps{l}_{p8}_{t0}_{sc}")
                            for a in range(2):
                                lh, rh = s_lhs_rhs(a, sc)
                                nc.tensor.matmul(
                                    pss[:, a, :], lhsT=lh, rhs=rh,
                                    start=True, stop=True)
                                if K_NO_PAIRS and prev is not None:
                                    av_mm_one(prev[0], prev[1], prev[2],
                                              pav, sc, a)
                            if K_SPLIT_EXP:
                                for a in range(2):
                                    nc.scalar.activation(
                                        et[:, sc, a, :], pss[:, a, :],
                                        AF.Exp, bias=0.0, scale=0.125)
                            else:
                                nc.scalar.activation(et[:, sc, :, :], pss,
                                                     AF.Exp, bias=0.0,
                                                     scale=0.125)
                            if not K_NO_PAIRS and prev is not None:
                                av_mm(prev[0], prev[1], prev[2], pav, sc)
                    if prev is not None:
                        av_out(prev[0], prev[1], pav)
                        if prev[1] + TB >= TW:
                            pass2(prev[0])
                    prev = (p8, t0, et)
            pav = ps.tile([65, 2, TB], f32, tag="avp", bufs=2,
                          name=f"pa{l}_{prev[0]}_{prev[1]}")
            for sc in range(CC):
                av_mm(prev[0], prev[1], prev[2], pav, sc)
            av_out(prev[0], prev[1], pav)
            pass2(prev[0])

            # proj + residual
            for mc in range(CC):
                ws = wsp.tile([128, CC, 128], bf16, tag="ws",
                              name=f"wp{l}_{mc}")
                nc.sync.dma_start(out=ws, in_=dr["wp"][l, mc])
                pm = ps.tile([128, 2, 512] if TW > 16 else [128, 2, 16],
                             f32, tag="s", bufs=2, name=f"pp{l}_{mc}")
                for t0 in range(0, TW, 512):
                    tw = min(512, TW - t0)
                    tp = t0 // 512
                    for kc in range(CC):
                        nc.tensor.matmul(pm[:, tp, :], lhsT=ws[:, kc, :],
                                         rhs=attn[:, kc, t0:t0 + tw],
                                         start=(kc == 0), stop=(kc == CC - 1))
                    nc.vector.scalar_tensor_tensor(
                        out=x[:, mc, t0:t0 + tw], in0=pm[:, tp, :],
                        scalar=biases["bp"][:, l, mc:mc + 1],
                        in1=x[:, mc, t0:t0 + tw], op0=Alu.add, op1=Alu.add)

        # ---- output: x[:, :, 0] ----
        o = cst.tile([128, CC], f32)
        nc.vector.tensor_copy(out=o, in_=x[:, :, 0:1].squeeze(-1))
        nc.sync.dma_start(out=out_d.rearrange("(c p) -> p c", p=128), in_=o)

    nc.compile()
    return nc


def _vtinit(bf):
    a = np.zeros((128, CC, 8, 2, 65), np.float32)
    a[:, :, :, :, 64] = 1.0
    return a.astype(bf)


def _prep(inputs):
    """Host-side weight restaging -> per-core input maps."""
    g = {k: np.asarray(v, np.float32) for k, v in inputs.items()}
    bf = ml_dtypes.bfloat16

    idx = np.arange(3 * C).reshape(H, 3, DH)
    qidx, kidx, vidx = idx[:, 0].ravel(), idx[:, 1].ravel(), idx[:, 2].ravel()

    def stage_w(w):            # w [NL, 1024(out), 1024(in)] -> staged lhsT
        wt = w.transpose(0, 2, 1)                    # [l, in, out]
        return np.ascontiguousarray(
            wt.reshape(NL, CC, 128, CC, 128).transpose(0, 3, 2, 1, 4)
        ).astype(bf)

    def stage_b(b):            # [NL, 1024] -> [128, NL, CC]
        return np.ascontiguousarray(
            b.reshape(NL, CC, 128).transpose(2, 0, 1))

    sel2 = np.zeros((2, 128), np.float32)
    for p in range(2):
        sel2[p, p * 64:(p + 1) * 64] = 1.0

    qkv_w, qkv_b = g["qkv_w"], g["qkv_b"]
    common = {
        "wq": stage_w(qkv_w[:, qidx, :]),
        "wk": stage_w(qkv_w[:, kidx, :]),
        "wv": stage_w(qkv_w[:, vidx, :]),
        "wp": stage_w(g["proj_w"]),
        "bq": stage_b(qkv_b[:, qidx]),
        "bk": stage_b(qkv_b[:, kidx]),
        "bv": stage_b(qkv_b[:, vidx]),
        "bp": stage_b(g["proj_b"]),
        "gnw": stage_b(g["gn_w"]),
        "gnb": stage_b(g["gn_b"]),
        "initw": np.ascontiguousarray(
            g["init_w"].T.reshape(SPEC, CC, 128)).astype(bf),
        "initb": np.ascontiguousarray(g["init_b"].reshape(CC, 128).T),
        "ind": np.equal(np.arange(128)[:, None] // 32,
                        np.arange(4)[None, :]).astype(np.float32),
        "indt": np.equal(np.arange(128)[None, :] // 32,
                         np.arange(4)[:, None]).astype(np.float32),
        "ident": np.eye(128, dtype=np.float32).astype(bf),
        "vtinit": _vtinit(bf),
        "sel2": sel2.astype(bf),
        "epsc": np.full((128, 1), EPS, np.float32),
    }
    in_maps = []
    for b in range(B):
        m = dict(common)
        m["speech"] = np.ascontiguousarray(g["speech"][b]).astype(bf)
        in_maps.append(m)
    return in_maps


def kernel(**inputs):
    global LAST_RESULT
    if "nc" not in _CACHE:
        _CACHE["nc"] = _build()
    nc = _CACHE["nc"]
    in_maps = _prep(inputs)
    res = run_bass_kernel_spmd(nc, in_maps, list(range(NCORES)))
    LAST_RESULT = res
    out = np.stack([res.results[b]["out"] for b in range(B)])
    return out.astype(np.float32)
